# revision 1
# baseline (speedup 1.0000x reference)
"""AttentionSubsample Trainium2 kernel.

Full (unsharded) inputs in, full output out. Data-parallel over batch:
32 batches -> 8 NeuronCores x 4 batches each. Weights/biases replicated.

Per-core dataflow (per batch element):
  kv proj   : kT[d,n] per head-pair + v[n,d] (psum -> sbuf with fused BN bias)
  q proj    : qT[d,q] per head-pair (fused BN bias + attn scale folded on host)
  per head  : scoresT[n,q] = kT.T @ qT (PE, K=64) in psum groups of 2 n-chunks,
              += relative-position bias (DVE/ACT), exp (ACT) -> E[n,q] sbuf,
              oT[d,q] += v.T @ E (PE), sums[q] += ones.T @ E (PE)
  normalize : oT *= 1/sums (DVE, fused psum->sbuf copy)
  hswish    : t = relu6(o+3); h = o*t    (1/6 folded into W_p on host)
  proj      : out[q,384] = h.T @ WpT + bp
Matmuls run as float32r (full PE rate at N>=256, fp32 storage); the
o-side (v, exp(scores), o, hswish, W_p) uses bf16 for 2x/4x DVE modes.
The relative-position bias is added on the PE (identity-matmul
accumulation into the scores PSUM) for ~10% of groups and on the DVE
for the rest, balancing engine load. The v-channel BN bias folds out
of the kernel entirely: softmax rows sum to 1, so attn@(v+bv) =
attn@v + bv, applied per-partition after normalization.
"""

import sys

if "/opt/trn_rl_repo" not in sys.path:
    sys.path.insert(0, "/opt/trn_rl_repo")

import ml_dtypes
import numpy as np

# --- problem constants (hardcoded, must match the grading reference) ---
B, N, C = 32, 1280, 256
H, KD, D = 8, 64, 128          # heads, key dim, value dim per head
NQ = 320                       # subsampled sequence length
OUT = 384
NCORES = 8
BPC = B // NCORES              # batches per core
EPS = 1e-5
NCH = N // 128                 # 10 n-chunks of 128
GRP = 2                        # scores psum group size (n-chunks per group)

_SUB_IDX = np.concatenate([
    (np.arange(32)[::2][:, None] * 32 + np.arange(32)[::2][None, :]).reshape(-1),
    1024 + (np.arange(16)[::2][:, None] * 16 + np.arange(16)[::2][None, :]).reshape(-1),
])  # [320] subsample row gather


def _prep(inputs):
    """Host-side: fold BN into weights, reorder channels, shard over cores."""
    f32 = np.float32
    x = np.asarray(inputs["x"], f32)
    g_kv, b_kv = np.asarray(inputs["g_kv"], f32), np.asarray(inputs["b_kv"], f32)
    rm_kv, rv_kv = np.asarray(inputs["rm_kv"], f32), np.asarray(inputs["rv_kv"], f32)
    g_q, b_q = np.asarray(inputs["g_q"], f32), np.asarray(inputs["b_q"], f32)
    rm_q, rv_q = np.asarray(inputs["rm_q"], f32), np.asarray(inputs["rv_q"], f32)
    g_p, b_p = np.asarray(inputs["g_p"], f32), np.asarray(inputs["b_p"], f32)
    rm_p, rv_p = np.asarray(inputs["rm_p"], f32), np.asarray(inputs["rv_p"], f32)
    W_kv = np.asarray(inputs["W_kv"], f32)
    W_q = np.asarray(inputs["W_q"], f32)
    W_p = np.asarray(inputs["W_p"], f32)
    attn_bias = np.asarray(inputs["attn_bias"], f32)
    bias_idxs = np.asarray(inputs["bias_idxs"])

    s_kv = g_kv / np.sqrt(rv_kv + EPS)
    Wkv_f = W_kv * s_kv[:, None]
    bkv_f = b_kv - rm_kv * s_kv
    kidx = np.concatenate([np.arange(h * 192, h * 192 + KD) for h in range(H)])
    vidx = np.concatenate([np.arange(h * 192 + KD, (h + 1) * 192) for h in range(H)])
    wkt = np.ascontiguousarray(Wkv_f[kidx].T).reshape(2, 128, 512)     # [c,128][512 kch]
    wvt = np.ascontiguousarray(Wkv_f[vidx].T).reshape(2, 128, 1024)
    bk = np.ascontiguousarray(bkv_f[kidx].reshape(4, 128).T)           # [128, 4]
    bvd = np.ascontiguousarray(bkv_f[vidx].reshape(8, 128).T)          # [128, H]

    scale = KD ** -0.5
    s_q = g_q / np.sqrt(rv_q + EPS)
    wqt = np.ascontiguousarray((W_q * (s_q * scale)[:, None]).T).reshape(2, 128, 512)
    bq = np.ascontiguousarray(((b_q - rm_q * s_q) * scale).reshape(4, 128).T)

    s_p = g_p / np.sqrt(rv_p + EPS)
    wpt = np.ascontiguousarray((W_p * s_p[:, None]).T / 6.0).reshape(
        8, 128, OUT).astype(ml_dtypes.bfloat16)
    bp = np.ascontiguousarray(np.broadcast_to(b_p - rm_p * s_p, (128, OUT)))

    biasT = attn_bias[:, bias_idxs].transpose(0, 2, 1)                 # [H, N, NQ]
    bias_d = np.ascontiguousarray(biasT.reshape(H, NCH, 128, NQ)).astype(
        ml_dtypes.bfloat16
    )

    xs = x[:, _SUB_IDX, :]                                             # [B, NQ, C]
    in_maps = []
    for i in range(NCORES):
        sl = slice(i * BPC, (i + 1) * BPC)
        xt = np.ascontiguousarray(x[sl].transpose(0, 2, 1)).reshape(BPC, 2, 128, N)
        xst = np.ascontiguousarray(xs[sl].transpose(0, 2, 1)).reshape(BPC, 2, 128, NQ)
        in_maps.append({
            "xt": xt, "xst": xst,
            "wkt": wkt, "wvt": wvt, "wqt": wqt, "wpt": wpt,
            "bk": bk, "bq": bq, "bv": bvd, "bp": bp,
            "bias": bias_d, "ones": np.ones((128, 128), ml_dtypes.bfloat16),
            "ident": np.eye(128, dtype=ml_dtypes.bfloat16),
        })
    return in_maps


def _body(tc, a, out_ap):
    import concourse.bass as bass  # noqa: F401
    import concourse.mybir as mybir
    from contextlib import ExitStack

    nc = tc.nc
    f32 = mybir.dt.float32
    f32r = mybir.dt.float32r
    bf16 = mybir.dt.bfloat16
    AF = mybir.ActivationFunctionType
    ALU = mybir.AluOpType

    def r(ap):
        return ap

    with ExitStack() as ctx:
        ctx.enter_context(
            nc.allow_low_precision(reason="o-side bf16 is deliberate; verified vs fp32 reference")
        )
        singles = ctx.enter_context(tc.tile_pool(name="singles", bufs=1))
        # DMA order matters at startup: small tiles + first-needed weights first
        bks = singles.tile([128, 4], f32)
        nc.sync.dma_start(bks, a["bk"])
        bqs = singles.tile([128, 4], f32)
        nc.sync.dma_start(bqs, a["bq"])
        ones = singles.tile([128, 128], bf16)
        nc.sync.dma_start(ones, a["ones"])
        ident = singles.tile([128, 128], bf16)
        nc.sync.dma_start(ident, a["ident"])
        wk = singles.tile([128, 2, 512], f32r)
        nc.sync.dma_start(wk, a["wkt"].rearrange("c p j -> p c j"))
        wq = singles.tile([128, 2, 512], f32r)
        nc.sync.dma_start(wq, a["wqt"].rearrange("c p j -> p c j"))
        wv = singles.tile([128, 2, 1024], f32r)
        bvs = singles.tile([128, H], f32)
        wp = singles.tile([128, 8, OUT], bf16)
        bps = singles.tile([128, OUT], f32)

        xt_p = ctx.enter_context(tc.tile_pool(name="xt", bufs=2))
        xst_p = ctx.enter_context(tc.tile_pool(name="xst", bufs=2))
        kt_p = ctx.enter_context(tc.tile_pool(name="kt", bufs=2))
        v_p = ctx.enter_context(tc.tile_pool(name="v", bufs=1))
        qt_p = ctx.enter_context(tc.tile_pool(name="qt", bufs=2))
        bias_p = ctx.enter_context(tc.tile_pool(name="bias", bufs=3))
        e_p = ctx.enter_context(tc.tile_pool(name="e", bufs=4))
        ot_p = ctx.enter_context(tc.tile_pool(name="ot", bufs=2))
        rc_p = ctx.enter_context(tc.tile_pool(name="rc", bufs=2))
        orw_p = ctx.enter_context(tc.tile_pool(name="orw", bufs=2))
        hs_p = ctx.enter_context(tc.tile_pool(name="hs", bufs=2))
        ob_p = ctx.enter_context(tc.tile_pool(name="ob", bufs=3))
        ps_work = ctx.enter_context(tc.tile_pool(name="ps_work", bufs=3, space="PSUM"))
        ps_o = ctx.enter_context(tc.tile_pool(name="ps_o", bufs=1, space="PSUM"))
        ps_sum = ctx.enter_context(tc.tile_pool(name="ps_sum", bufs=1, space="PSUM"))

        _wt_n = [0]

        def work_tile():
            _wt_n[0] += 1
            return ps_work.tile([128, GRP, 512], f32, tag="w", name=f"wt{_wt_n[0]}")

        for b in range(BPC):
            xt = xt_p.tile([128, 2, N], f32r)
            for ns in range(3):
                n0 = ns * 512
                nsz = min(512, N - n0)
                nc.sync.dma_start(
                    xt[:, :, n0:n0 + nsz],
                    a["xt"][b, :, :, n0:n0 + nsz].rearrange("c p n -> p c n"),
                )
            xst = xst_p.tile([128, 2, NQ], f32r)
            nc.sync.dma_start(xst, a["xst"][b].rearrange("c p n -> p c n"))
            if b == 0:
                nc.sync.dma_start(wv, a["wvt"].rearrange("c p j -> p c j"))
                nc.sync.dma_start(bvs, a["bv"])
                nc.sync.dma_start(wp, a["wpt"].rearrange("c p j -> p c j"))
                nc.sync.dma_start(bps, a["bp"])

            kt = kt_p.tile([128, 4, N], f32r)    # [d(2 heads), pair, n]
            vt = v_p.tile([128, NCH, 1024], bf16)  # [n, chunk, v-ch head-major]
            qt = qt_p.tile([128, 4, NQ], f32r)   # [d(2 heads), pair, q]

            # --- kv/q projections ---
            for pr in range(4):                 # kT: head pairs
                for ns in range(3):             # n slices 512/512/256
                    n0 = ns * 512
                    nsz = min(512, N - n0)
                    ps = work_tile()[:, 0, :]
                    for cc in range(2):
                        nc.tensor.matmul(
                            ps[:, :nsz],
                            lhsT=r(wk[:, cc, pr * 128:(pr + 1) * 128]),
                            rhs=r(xt[:, cc, n0:n0 + nsz]),
                            start=(cc == 0), stop=(cc == 1),
                        )
                    nc.scalar.activation(
                        kt[:, pr, n0:n0 + nsz], ps[:, :nsz],
                        AF.Identity, bias=bks[:, pr:pr + 1],
                    )
            for cn in range(NCH):               # v: [n-chunk, 4 heads' v] x2
                for hf in range(2):
                    ps = work_tile()[:, 0, :]
                    for cc in range(2):
                        nc.tensor.matmul(
                            ps,
                            lhsT=r(xt[:, cc, cn * 128:(cn + 1) * 128]),
                            rhs=r(wv[:, cc, hf * 512:(hf + 1) * 512]),
                            start=(cc == 0), stop=(cc == 1),
                        )
                    if (cn + hf) % 2 == 0:
                        nc.scalar.copy(vt[:, cn, hf * 512:(hf + 1) * 512], ps)
                    else:
                        nc.vector.tensor_copy(vt[:, cn, hf * 512:(hf + 1) * 512], ps)
            for pr in range(4):                 # qT
                ps = work_tile()[:, 0, :]
                for cc in range(2):
                    nc.tensor.matmul(
                        ps[:, :NQ],
                        lhsT=r(wq[:, cc, pr * 128:(pr + 1) * 128]),
                        rhs=r(xst[:, cc, :]),
                        start=(cc == 0), stop=(cc == 1),
                    )
                nc.vector.tensor_tensor(
                    qt[:, pr, :], ps[:, :NQ],
                    bqs[:, pr:pr + 1].to_broadcast((128, NQ)), ALU.add,
                )

            # --- attention per head ---
            ot = ot_p.tile([128, H, NQ], bf16)  # [d, head, q]
            if b % 2 == 0:
                # proj lhsT for a batch PAIR: q=640 = 5x128 exact (vs 2.5x128)
                t2 = hs_p.tile([128, H, 2, NQ], bf16, tag="t2", name=f"t2_{b}")
            t = t2[:, :, b % 2, :]
            for h in range(H):
                pr, p0 = h // 2, 64 * (h % 2)
                bt = bias_p.tile([128, NCH, NQ], bf16)
                nc.sync.dma_start(bt, a["bias"][h].rearrange("c p q -> p c q"))
                po = ps_o.tile([128, NQ], f32)
                psm = ps_sum.tile([128, NQ], f32)
                for g in range(NCH // GRP):
                    # bias-add engine: mostly DVE (PE is the busiest engine);
                    # every 5th group keeps the identity-matmul PE path.
                    on_pe = (h * (NCH // GRP) + g) % 10 == 9
                    sg = work_tile()
                    for j in range(GRP):
                        c = GRP * g + j
                        nc.tensor.matmul(
                            sg[:, j, :NQ],
                            lhsT=r(kt[p0:p0 + 64, pr, c * 128:(c + 1) * 128]),
                            rhs=r(qt[p0:p0 + 64, pr, :]),
                            start=True, stop=(not on_pe),
                        )
                        if on_pe:
                            nc.tensor.matmul(
                                sg[:, j, :NQ],
                                lhsT=ident,
                                rhs=bt[:, c, :],
                                start=False, stop=True,
                            )
                    if not on_pe:
                        nc.vector.tensor_tensor(
                            sg[:, :, :NQ], sg[:, :, :NQ],
                            bt[:, GRP * g:GRP * (g + 1), :], ALU.add,
                        )
                    e = e_p.tile([128, GRP, NQ], bf16)
                    nc.scalar.activation(e, sg[:, :, :NQ], AF.Exp)
                    for j in range(GRP):
                        c = GRP * g + j
                        nc.tensor.matmul(
                            po,
                            lhsT=r(vt[:, c, h * 128:(h + 1) * 128]),
                            rhs=r(e[:, j, :]),
                            start=(c == 0), stop=(c == NCH - 1),
                        )
                        nc.tensor.matmul(
                            psm,
                            lhsT=r(ones),
                            rhs=r(e[:, j, :]),
                            start=(c == 0), stop=(c == NCH - 1),
                        )
                # free the po/psm banks ASAP: raw copies on ACT, then the
                # normalize runs from SBUF off the PE-critical path
                oraw = orw_p.tile([128, NQ], bf16, tag="oraw", name=f"oraw{b}_{h}")
                nc.scalar.copy(oraw, po)
                rc = rc_p.tile([128, NQ], bf16)
                nc.vector.reciprocal(rc, psm)
                oh = ot[:, h, :]
                # o = oraw/sums + bv  (bv folds out of A-v: softmax rows sum to 1)
                nc.vector.tensor_tensor(oh, oraw, rc, ALU.mult)
                nc.vector.tensor_scalar_add(oh, oh, bvs[:, h:h + 1])
                th = t[:, h, :]
                nc.any.tensor_scalar(th, oh, 3.0, 6.0, ALU.add, ALU.min)
                nc.any.tensor_scalar(th, th, 0.0, None, ALU.max)
                nc.any.tensor_tensor(th, th, oh, ALU.mult)

            # --- output projection (per batch pair, q merged to 640) ---
            if b % 2 == 1:
                out_flat = out_ap.rearrange("b q o -> (b q) o")
                for qc in range(5):
                    r0 = (b - 1) * NQ + qc * 128
                    ps = work_tile()[:, 0, :]
                    for dc in range(8):
                        nc.tensor.matmul(
                            ps[:, :OUT],
                            lhsT=t2[:, dc, :, :].rearrange(
                                "p bb q -> p (bb q)")[:, qc * 128:(qc + 1) * 128],
                            rhs=r(wp[:, dc, :]),
                            start=(dc == 0), stop=(dc == 7),
                        )
                    ob = ob_p.tile([128, OUT], f32)
                    nc.any.tensor_tensor(ob, ps[:, :OUT], bps, ALU.add)
                    nc.sync.dma_start(out_flat[r0:r0 + 128, :], ob)


def build():
    import concourse.mybir as mybir
    import concourse.tile as tile
    from concourse import bacc

    nc = bacc.Bacc("TRN2", target_bir_lowering=False, debug=False)
    f32, bf16 = mybir.dt.float32, mybir.dt.bfloat16
    a = {}

    def din(name, shape, dt=f32):
        a[name] = nc.dram_tensor(name, shape, dt, kind="ExternalInput").ap()

    f32r = mybir.dt.float32r
    din("xt", [BPC, 2, 128, N], f32r)
    din("xst", [BPC, 2, 128, NQ], f32r)
    din("wkt", [2, 128, 512], f32r)
    din("wvt", [2, 128, 1024], f32r)
    din("wqt", [2, 128, 512], f32r)
    din("wpt", [8, 128, OUT], bf16)
    din("bk", [128, 4])
    din("bq", [128, 4])
    din("bv", [128, H])
    din("bp", [128, OUT])
    din("bias", [H, NCH, 128, NQ], bf16)
    din("ones", [128, 128], bf16)
    din("ident", [128, 128], bf16)
    out_ap = nc.dram_tensor("out", [BPC, NQ, OUT], f32, kind="ExternalOutput").ap()

    with tile.TileContext(nc) as tc:
        _body(tc, a, out_ap)
    nc.compile()
    return nc


_NC_CACHE = None


def _get_nc():
    global _NC_CACHE
    if _NC_CACHE is None:
        _NC_CACHE = build()
    return _NC_CACHE


def kernel(**inputs):
    from concourse.bass_utils import run_bass_kernel_spmd

    in_maps = _prep(inputs)
    nc = _get_nc()
    res = run_bass_kernel_spmd(nc, in_maps, list(range(NCORES)))
    out = np.concatenate([res.results[i]["out"] for i in range(NCORES)], axis=0)
    return np.ascontiguousarray(out, dtype=np.float32)


if __name__ == "__main__":
    rng = np.random.default_rng(0)
    print("smoke: building bass module...")
    nc = build()
    print("built ok:", sum(len(bb.instructions) for bb in nc.m.functions[0].blocks), "instructions")



# revision 4
# speedup vs baseline: 1.2526x; 1.2526x over previous
"""AttentionSubsample Trainium2 kernel.

Full (unsharded) inputs in, full output out. Data-parallel over batch:
32 batches -> 8 NeuronCores x 4 batches each. Weights/biases replicated.

Engine-balance design (cost-model 219us/core, vs 275us v1 baseline):
  - k-channel BN bias dropped entirely: softmax over n is invariant to
    per-q shifts and (k+bk)@q shifts every key n equally.
  - score bias added pre-exp on the PE as fp8(e4m3) DoubleRow identity
    matmuls (0.5 cyc/row): lhsT=(I,0)/(0,I) selects one chunk of an
    adjacent bias-chunk pair, so the bias stays resident in SBUF stored
    once (3.2KB/partition/head, loaded one time, no per-batch DMA).
  - v projection as fp8 hi/lo split (x = x8h + x8l, Wv = w8h + w8l) with
    three K=256 DoubleRow passes per psum tile, dropping the lo*lo term:
    ~2.7x fewer PE cycles than f32r at bf16-level accuracy. kT/q stay
    f32r: their quantization noise would amplify through exp by sqrt(d).
  - softmax sums: e-tiles accumulated on DVE (bf16 2x mode, in-place
    chain) + one ones-matmul per head instead of 10 PE ones-matmuls.
  - hswish on Pool/DVE: t = min(Relu(o+3+bv), 6) via Pool tensor_scalar
    ops, th = (o+bv)*t via Pool scalar_tensor_tensor; normalize mult on
    Pool; bv folds out of attn@v (softmax rows sum to 1).
  - psum->sbuf copies split across ACT/DVE (GPSIMD cannot touch PSUM on
    real hw); out-proj bias fused into the DVE psum->sbuf add.
  - software pipelining: batch b+1's kT/q/v projection psum tiles are
    emitted interleaved between batch b's attention heads (2-3 tiles per
    head), and the pair output projection interleaves with the following
    batch, keeping the PE fed through the shared psum-pool rotation.
  - PSUM: scores pool 3x[128,2,512] (chunk pairs at bank-aligned 512
    offsets, exp reads the [*, :320] pair in one ACT instr), po + psm
    1 bank each = 8 banks.
"""

import sys

if "/opt/trn_rl_repo" not in sys.path:
    sys.path.insert(0, "/opt/trn_rl_repo")

import ml_dtypes
import numpy as np

# --- problem constants (hardcoded, must match the grading reference) ---
B, N, C = 32, 1280, 256
H, KD, D = 8, 64, 128          # heads, key dim, value dim per head
NQ = 320                       # subsampled sequence length
OUT = 384
NCORES = 8
BPC = B // NCORES              # batches per core
EPS = 1e-5
NCH = N // 128                 # 10 n-chunks of 128
GRP = 2                        # scores psum group size (n-chunks per group)

# per-head engine tuning: bias add on PE (fp8 DoubleRow) vs DVE (exp-bias mult)
BIAS_PE = [True] * 8
# per-head: softmax sums via 10 PE ones-matmuls vs DVE accumulate + 1 matmul
SUMS_PE = [False] * 8

_PE_HEADS = [h for h in range(H) if BIAS_PE[h]]
_DVE_HEADS = [h for h in range(H) if not BIAS_PE[h]]
_PE_SLOT = {h: i for i, h in enumerate(_PE_HEADS)}
_DVE_SLOT = {h: i for i, h in enumerate(_DVE_HEADS)}

_SUB_IDX = np.concatenate([
    (np.arange(32)[::2][:, None] * 32 + np.arange(32)[::2][None, :]).reshape(-1),
    1024 + (np.arange(16)[::2][:, None] * 16 + np.arange(16)[::2][None, :]).reshape(-1),
])  # [320] subsample row gather


def _prep(inputs):
    """Host-side: fold BN into weights, reorder channels, shard over cores."""
    f32 = np.float32
    x = np.asarray(inputs["x"], f32)
    g_kv, b_kv = np.asarray(inputs["g_kv"], f32), np.asarray(inputs["b_kv"], f32)
    rm_kv, rv_kv = np.asarray(inputs["rm_kv"], f32), np.asarray(inputs["rv_kv"], f32)
    g_q, b_q = np.asarray(inputs["g_q"], f32), np.asarray(inputs["b_q"], f32)
    rm_q, rv_q = np.asarray(inputs["rm_q"], f32), np.asarray(inputs["rv_q"], f32)
    g_p, b_p = np.asarray(inputs["g_p"], f32), np.asarray(inputs["b_p"], f32)
    rm_p, rv_p = np.asarray(inputs["rm_p"], f32), np.asarray(inputs["rv_p"], f32)
    W_kv = np.asarray(inputs["W_kv"], f32)
    W_q = np.asarray(inputs["W_q"], f32)
    W_p = np.asarray(inputs["W_p"], f32)
    attn_bias = np.asarray(inputs["attn_bias"], f32)
    bias_idxs = np.asarray(inputs["bias_idxs"])

    s_kv = g_kv / np.sqrt(rv_kv + EPS)
    Wkv_f = W_kv * s_kv[:, None]
    bkv_f = b_kv - rm_kv * s_kv
    kidx = np.concatenate([np.arange(h * 192, h * 192 + KD) for h in range(H)])
    vidx = np.concatenate([np.arange(h * 192 + KD, (h + 1) * 192) for h in range(H)])
    wkt = np.ascontiguousarray(Wkv_f[kidx].T).reshape(2, 128, 512)     # [c,128][512 kch]
    wvt = np.ascontiguousarray(Wkv_f[vidx].T).reshape(2, 128, 1024)
    bvd = np.ascontiguousarray(bkv_f[vidx].reshape(8, 128).T)          # [128, H]

    scale = KD ** -0.5
    s_q = g_q / np.sqrt(rv_q + EPS)
    wqt = np.ascontiguousarray((W_q * (s_q * scale)[:, None]).T).reshape(2, 128, 512)
    bq = np.ascontiguousarray(((b_q - rm_q * s_q) * scale).reshape(4, 128).T)

    s_p = g_p / np.sqrt(rv_p + EPS)
    wpt = np.ascontiguousarray((W_p * s_p[:, None]).T / 6.0).reshape(
        8, 128, OUT).astype(ml_dtypes.bfloat16)
    bps = np.ascontiguousarray(np.broadcast_to(b_p - rm_p * s_p, (128, OUT))).astype(np.float32)

    biasT = attn_bias[:, bias_idxs].transpose(0, 2, 1)                 # [H, N, NQ]
    bias_cpq = biasT.reshape(H, NCH, 128, NQ).transpose(0, 2, 1, 3)    # [H,128,NCH,NQ]
    f8 = ml_dtypes.float8_e4m3
    # bias fp8, stored once per head; the DoubleRow identity pair (I,0)/(0,I)
    # selects one chunk of an adjacent pair per instruction
    bt8 = np.ascontiguousarray(bias_cpq).astype(f8)                    # [H,128,NCH,NQ]

    identp = np.zeros((128, 2, 2, 128), f8)
    identp[np.arange(128), 0, 0, np.arange(128)] = 1.0
    identp[np.arange(128), 1, 1, np.arange(128)] = 1.0

    wv8h = wvt.astype(f8)
    wv8l = (wvt - wv8h.astype(np.float32)).astype(f8)

    xs = x[:, _SUB_IDX, :]                                             # [B, NQ, C]
    in_maps = []
    for i in range(NCORES):
        sl = slice(i * BPC, (i + 1) * BPC)
        xt = np.ascontiguousarray(x[sl].transpose(0, 2, 1)).reshape(BPC, 2, 128, N)
        x8h = xt.astype(f8)
        x8l = (xt - x8h.astype(np.float32)).astype(f8)
        xst = np.ascontiguousarray(xs[sl].transpose(0, 2, 1)).reshape(BPC, 2, 128, NQ)
        in_maps.append({
            "xt": xt, "xst": xst, "x8h": x8h, "x8l": x8l,
            "wv8h": wv8h, "wv8l": wv8l,
            "wkt": wkt, "wvt": wvt, "wqt": wqt, "wpt": wpt,
            "bq": bq, "bv": bvd, "bv3": bvd + 3.0, "bps": bps,
            "bt8": bt8,
            "ones": np.ones((128, 128), ml_dtypes.bfloat16),
            "identp": identp,
        })
    return in_maps


def _body(tc, a, out_ap):
    import concourse.bass as bass  # noqa: F401
    import concourse.mybir as mybir
    from contextlib import ExitStack

    nc = tc.nc
    f32 = mybir.dt.float32
    f32r = mybir.dt.float32r
    bf16 = mybir.dt.bfloat16
    f8e4 = mybir.dt.float8e4
    AF = mybir.ActivationFunctionType
    ALU = mybir.AluOpType
    PM = mybir.MatmulPerfMode

    with ExitStack() as ctx:
        ctx.enter_context(
            nc.allow_low_precision(reason="bf16 o-side + fp8 bias matmuls are deliberate; verified vs fp32 reference")
        )
        singles = ctx.enter_context(tc.tile_pool(name="singles", bufs=1))
        # DMA order matters at startup: first-needed weights first (wk -> q/kT
        # projections of batch 0), small attention-phase tiles later.
        wk = singles.tile([128, 2, 512], f32r)
        nc.sync.dma_start(wk[:, :, 0:128], a["wkt"][:, :, 0:128].rearrange("c p j -> p c j"))
        wq = singles.tile([128, 2, 512], f32r)
        bqs = singles.tile([128, 4], f32)
        wv8h = singles.tile([128, 2, 1024], f8e4)
        wv8l = singles.tile([128, 2, 1024], f8e4)
        wp = singles.tile([128, 8, OUT], bf16)
        bvs = singles.tile([128, H], f32)
        bvs3 = singles.tile([128, H], f32)
        ones = singles.tile([128, 128], bf16)
        identp = singles.tile([128, 2, 2, 128], f8e4)
        bps = singles.tile([128, OUT], f32)
        bt8s = [singles.tile([128, NCH, NQ], f8e4, name=f"bt8h{h}")
                for h in range(H)]

        xt_p = ctx.enter_context(tc.tile_pool(name="xt", bufs=2))
        xst_p = ctx.enter_context(tc.tile_pool(name="xst", bufs=2))
        x8_p = ctx.enter_context(tc.tile_pool(name="x8", bufs=2))
        kt_p = ctx.enter_context(tc.tile_pool(name="kt", bufs=2))
        v_p = ctx.enter_context(tc.tile_pool(name="v", bufs=2))
        qt_p = ctx.enter_context(tc.tile_pool(name="qt", bufs=2))
        e_p = ctx.enter_context(tc.tile_pool(name="e", bufs=6))
        esum_p = ctx.enter_context(tc.tile_pool(name="esum", bufs=1))
        orw_p = ctx.enter_context(tc.tile_pool(name="orw", bufs=2))
        rc_p = ctx.enter_context(tc.tile_pool(name="rc", bufs=2))
        oh_p = ctx.enter_context(tc.tile_pool(name="oh", bufs=2))
        t1_p = ctx.enter_context(tc.tile_pool(name="t1", bufs=2))
        hs_p = ctx.enter_context(tc.tile_pool(name="hs", bufs=2))
        ob_p = ctx.enter_context(tc.tile_pool(name="ob", bufs=2))
        ps_sg = ctx.enter_context(tc.tile_pool(name="ps_sg", bufs=3, space="PSUM"))
        ps_o = ctx.enter_context(tc.tile_pool(name="ps_o", bufs=1, space="PSUM"))
        ps_sum = ctx.enter_context(tc.tile_pool(name="ps_sum", bufs=1, space="PSUM"))

        _wt_n = [0]

        def sg_tile():
            _wt_n[0] += 1
            return ps_sg.tile([128, GRP, 512], f32, tag="sg", name=f"sg{_wt_n[0]}")

        out_flat = out_ap.rearrange("b q o -> (b q) o")

        def dma_x(b):
            """Issue input DMAs for batch b; returns (xt, xst) tiles."""
            xt = xt_p.tile([128, 2, N], f32r, tag="xt", name=f"xt{b}")
            for ns in range(3):
                n0 = ns * 512
                nsz = min(512, N - n0)
                nc.sync.dma_start(
                    xt[:, :, n0:n0 + nsz],
                    a["xt"][b, :, :, n0:n0 + nsz].rearrange("c p n -> p c n"),
                )
            xst = xst_p.tile([128, 2, NQ], f32r, tag="xst", name=f"xst{b}")
            nc.sync.dma_start(xst, a["xst"][b].rearrange("c p n -> p c n"))
            x8h = x8_p.tile([128, 2, N], f8e4, tag="x8h", name=f"x8h{b}")
            nc.sync.dma_start(x8h, a["x8h"][b].rearrange("c p n -> p c n"))
            x8l = x8_p.tile([128, 2, N], f8e4, tag="x8l", name=f"x8l{b}")
            nc.sync.dma_start(x8l, a["x8l"][b].rearrange("c p n -> p c n"))
            return xt, xst, x8h, x8l

        def proj_gen(b, xt, xst, x8h, x8l):
            """Yield after each proj psum tile; returns (kt, vt, qt) eagerly."""
            kt = kt_p.tile([128, 4, N], f32r, tag="kt", name=f"kt{b}")
            vt = v_p.tile([128, NCH, 1024], bf16, tag="vt", name=f"vt{b}")
            qt = qt_p.tile([128, 4, NQ], f32r, tag="qt", name=f"qt{b}")

            def emit():
                # kT projection: no bias (softmax-invariant), wide copies
                for pr in range(4):
                    ps = sg_tile()
                    for half in range(2):       # n slices 0:512, 512:1024
                        n0 = half * 512
                        for cc in range(2):
                            nc.tensor.matmul(
                                ps[:, half, :],
                                lhsT=wk[:, cc, pr * 128:(pr + 1) * 128],
                                rhs=xt[:, cc, n0:n0 + 512],
                                start=(cc == 0), stop=(cc == 1),
                            )
                    if pr < 2:
                        nc.vector.tensor_copy(
                            kt[:, pr, 0:1024], ps.rearrange("p g j -> p (g j)"),
                        )
                    else:
                        nc.scalar.copy(
                            kt[:, pr, 0:1024], ps.rearrange("p g j -> p (g j)"),
                        )
                    yield
                ps = sg_tile()                  # 256-col tails, two prs per tile
                for prh in range(2):
                    for j in range(2):
                        pr = 2 * prh + j
                        for cc in range(2):
                            nc.tensor.matmul(
                                ps[:, j, :256] if prh == 0 else ps[:, j, 256:512],
                                lhsT=wk[:, cc, pr * 128:(pr + 1) * 128],
                                rhs=xt[:, cc, 1024:N],
                                start=(cc == 0), stop=(cc == 1),
                            )
                        if prh == 0:
                            nc.scalar.copy(kt[:, pr, 1024:N], ps[:, j, :256])
                        else:
                            nc.scalar.copy(kt[:, pr, 1024:N], ps[:, j, 256:512])
                yield
                # q projection: 2 prs per tile, fused bias on DVE
                for half in range(2):
                    ps = sg_tile()
                    for j in range(2):
                        pr = 2 * half + j
                        for cc in range(2):
                            nc.tensor.matmul(
                                ps[:, j, :NQ],
                                lhsT=wq[:, cc, pr * 128:(pr + 1) * 128],
                                rhs=xst[:, cc, :],
                                start=(cc == 0), stop=(cc == 1),
                            )
                    nc.vector.tensor_tensor(
                        qt[:, 2 * half:2 * half + 2, :], ps[:, :, :NQ],
                        bqs[:, 2 * half:2 * half + 2].to_broadcast((128, 2, NQ)),
                        ALU.add,
                    )
                    yield
                # v projection: fp8 hi/lo DoubleRow (K=256 per pass, 3 passes)
                for cn in range(NCH):
                    ps = sg_tile()
                    for hf in range(2):
                        for pi, (xx, ww) in enumerate(
                                ((x8h, wv8h), (x8l, wv8h), (x8h, wv8l))):
                            nc.tensor.matmul(
                                ps[:, hf, :],
                                lhsT=xx[:, :, cn * 128:(cn + 1) * 128],
                                rhs=ww[:, :, hf * 512:(hf + 1) * 512],
                                start=(pi == 0), stop=(pi == 2),
                                perf_mode=PM.DoubleRow,
                            )
                    if cn % 2 == 0:
                        nc.vector.tensor_copy(
                            vt[:, cn, :], ps.rearrange("p g j -> p (g j)"))
                    else:
                        nc.scalar.copy(
                            vt[:, cn, :], ps.rearrange("p g j -> p (g j)"))
                    yield

            return kt, vt, qt, emit()

        def attention(b, h, kt, vt, qt, t2):
            pr, p0 = h // 2, 64 * (h % 2)
            on_pe = BIAS_PE[h]
            bt8 = bt8s[h]
            po = ps_o.tile([128, NQ], f32, tag="po", name=f"po_{b}_{h}")
            e_tiles = []
            for g in range(NCH // GRP):
                sg = sg_tile()
                for j in range(GRP):
                    c = GRP * g + j
                    nc.tensor.matmul(
                        sg[:, j, :NQ],
                        lhsT=kt[p0:p0 + 64, pr, c * 128:(c + 1) * 128],
                        rhs=qt[p0:p0 + 64, pr, :],
                        start=True, stop=(not on_pe),
                    )
                    if on_pe:
                        nc.tensor.matmul(
                            sg[:, j, :NQ],
                            lhsT=identp[:, j, :, :],
                            rhs=bt8[:, GRP * g:GRP * (g + 1), :],
                            start=False, stop=True,
                            perf_mode=PM.DoubleRow,
                        )
                e = e_p.tile([128, GRP, NQ], bf16)
                nc.scalar.activation(e, sg[:, :, :NQ], AF.Exp)
                e_tiles.append(e)
                for j in range(GRP):
                    c = GRP * g + j
                    nc.tensor.matmul(
                        po[:, :NQ],
                        lhsT=vt[:, c, h * 128:(h + 1) * 128],
                        rhs=e[:, j, :],
                        start=(c == 0), stop=(c == NCH - 1),
                    )
            # softmax denominators
            psm = ps_sum.tile([128, NQ], f32, tag="psm", name=f"psm_{b}_{h}")
            if SUMS_PE[h]:
                for g in range(NCH // GRP):
                    for j in range(GRP):
                        c = GRP * g + j
                        nc.tensor.matmul(
                            psm,
                            lhsT=ones,
                            rhs=e_tiles[g][:, j, :],
                            start=(c == 0), stop=(c == NCH - 1),
                        )
            else:
                acc = e_tiles[1]
                nc.vector.tensor_tensor(acc, e_tiles[0], e_tiles[1], ALU.add)
                for g in range(2, NCH // GRP):
                    nc.vector.tensor_tensor(acc, acc, e_tiles[g], ALU.add)
                esum = esum_p.tile([128, NQ], bf16)
                nc.vector.tensor_tensor(esum, acc[:, 0, :], acc[:, 1, :], ALU.add)
                nc.tensor.matmul(psm, lhsT=ones, rhs=esum, start=True, stop=True)
            rc = rc_p.tile([128, NQ], bf16)
            nc.vector.reciprocal(rc, psm)
            oraw = orw_p.tile([128, NQ], bf16, tag="oraw", name=f"oraw_{b}_{h}")
            nc.vector.tensor_copy(oraw, po)
            oh = oh_p.tile([128, NQ], bf16)
            nc.gpsimd.tensor_tensor(oh, oraw, rc, ALU.mult)
            # hswish: t = min(Relu(o + 3 + bv), 6);  th = (o + bv) * t
            t1 = t1_p.tile([128, NQ], bf16)
            nc.gpsimd.tensor_scalar(t1, oh, bvs3[:, h:h + 1], 0.0, ALU.add, ALU.max)
            nc.gpsimd.tensor_scalar(t1, t1, 6.0, None, ALU.min)
            obv = t1_p.tile([128, NQ], bf16, tag="obv", name=f"obv_{b}_{h}")
            nc.gpsimd.tensor_scalar(obv, oh, bvs[:, h:h + 1], None, ALU.add)
            nc.gpsimd.tensor_tensor(t2[:, h, b % 2, :], obv, t1, ALU.mult)

        def out_proj(b, t2):
            for qc in range(5):
                r0 = (b - 1) * NQ + qc * 128
                ps = sg_tile()
                for dc in range(8):
                    nc.tensor.matmul(
                        ps[:, 0, :OUT],
                        lhsT=t2[:, dc, :, :].rearrange(
                            "p bb q -> p (bb q)")[:, qc * 128:(qc + 1) * 128],
                        rhs=wp[:, dc, :],
                        start=(dc == 0), stop=(dc == 7),
                    )
                ob = ob_p.tile([128, OUT], f32)
                nc.vector.tensor_tensor(ob, ps[:, 0, :OUT], bps, ALU.add)
                nc.sync.dma_start(out_flat[r0:r0 + 128, :], ob)
                yield

        # prologue: batch 0 inputs + weights; proj(0) up to attention-ready
        xt0, xst0, x8h0, x8l0 = dma_x(0)
        nc.sync.dma_start(wk[:, :, 128:512], a["wkt"][:, :, 128:512].rearrange("c p j -> p c j"))
        nc.sync.dma_start(wq, a["wqt"].rearrange("c p j -> p c j"))
        nc.sync.dma_start(bqs, a["bq"])
        nc.sync.dma_start(wv8h, a["wv8h"].rearrange("c p j -> p c j"))
        nc.sync.dma_start(wv8l, a["wv8l"].rearrange("c p j -> p c j"))
        nc.sync.dma_start(ones, a["ones"])
        nc.sync.dma_start(identp, a["identp"])
        nc.sync.dma_start(bvs, a["bv"])
        nc.sync.dma_start(bvs3, a["bv3"])
        nc.sync.dma_start(wp, a["wpt"].rearrange("c p j -> p c j"))
        nc.sync.dma_start(bps, a["bps"])
        for h in range(H):
            nc.sync.dma_start(bt8s[h], a["bt8"][h])

        kt, vt, qt, gen0 = proj_gen(0, xt0, xst0, x8h0, x8l0)
        for _ in gen0:          # batch 0 proj must fully precede its attention
            pass
        pending = []
        t2 = None
        nxt = None
        for b in range(BPC):
            if b % 2 == 0:
                t2 = hs_p.tile([128, H, 2, NQ], bf16, tag="t2", name=f"t2_{b}")
            # interleave remaining proj tiles (this batch's tail + next batch)
            if b + 1 < BPC:
                xtn, xstn, x8hn, x8ln = dma_x(b + 1)
                nxt = proj_gen(b + 1, xtn, xstn, x8hn, x8ln)
                pending.append(nxt[3])
            for h in range(H):
                attention(b, h, kt, vt, qt, t2)
                for _ in range(3 if h >= 5 else 2):
                    while pending:
                        if next(pending[0], "done") == "done":
                            pending.pop(0)
                        else:
                            break
            while pending:
                if next(pending[0], "done") == "done":
                    pending.pop(0)
                else:
                    break
            if pending:
                for _ in pending[0]:
                    pass
                pending.pop(0)
            if b % 2 == 1:
                og = out_proj(b, t2)
                if b + 1 < BPC:
                    pending.append(og)   # interleave with next batch's heads
                else:
                    for _ in og:
                        pass
            if nxt is not None:
                kt, vt, qt = nxt[0], nxt[1], nxt[2]
                nxt = None


def build():
    import concourse.mybir as mybir
    import concourse.tile as tile
    from concourse import bacc

    nc = bacc.Bacc("TRN2", target_bir_lowering=False, debug=False)
    f32, bf16 = mybir.dt.float32, mybir.dt.bfloat16
    f8e4 = mybir.dt.float8e4
    a = {}

    def din(name, shape, dt=f32):
        a[name] = nc.dram_tensor(name, shape, dt, kind="ExternalInput").ap()

    f32r = mybir.dt.float32r
    din("xt", [BPC, 2, 128, N], f32r)
    din("xst", [BPC, 2, 128, NQ], f32r)
    din("wkt", [2, 128, 512], f32r)
    din("wvt", [2, 128, 1024], f32r)
    din("x8h", [BPC, 2, 128, N], f8e4)
    din("x8l", [BPC, 2, 128, N], f8e4)
    din("wv8h", [2, 128, 1024], f8e4)
    din("wv8l", [2, 128, 1024], f8e4)
    din("wqt", [2, 128, 512], f32r)
    din("wpt", [8, 128, OUT], bf16)
    din("bq", [128, 4])
    din("bv", [128, H])
    din("bv3", [128, H])
    din("bps", [128, OUT])
    din("bt8", [H, 128, NCH, NQ], f8e4)
    din("ones", [128, 128], bf16)
    din("identp", [128, 2, 2, 128], f8e4)
    out_ap = nc.dram_tensor("out", [BPC, NQ, OUT], f32, kind="ExternalOutput").ap()

    with tile.TileContext(nc) as tc:
        _body(tc, a, out_ap)
    nc.compile()
    return nc


_NC_CACHE = None


def _get_nc():
    global _NC_CACHE
    if _NC_CACHE is None:
        _NC_CACHE = build()
    return _NC_CACHE


def kernel(**inputs):
    from concourse.bass_utils import run_bass_kernel_spmd

    in_maps = _prep(inputs)
    nc = _get_nc()
    res = run_bass_kernel_spmd(nc, in_maps, list(range(NCORES)))
    out = np.concatenate([res.results[i]["out"] for i in range(NCORES)], axis=0)
    return np.ascontiguousarray(out, dtype=np.float32)


if __name__ == "__main__":
    rng = np.random.default_rng(0)
    print("smoke: building bass module...")
    nc = build()
    print("built ok:", sum(len(bb.instructions) for bb in nc.m.functions[0].blocks), "instructions")


# revision 5
# speedup vs baseline: 1.2572x; 1.0037x over previous
"""AttentionSubsample Trainium2 kernel.

Full (unsharded) inputs in, full output out. Data-parallel over batch:
32 batches -> 8 NeuronCores x 4 batches each. Weights/biases replicated.

Engine-balance design (cost-model 219.2us/core, vs 275.6us v1 baseline):
  - k-channel BN bias dropped entirely: softmax over n is invariant to
    per-q shifts and (k+bk)@q shifts every key n equally.
  - score bias added pre-exp on the PE as fp8(e4m3) DoubleRow identity
    matmuls (0.5 cyc/row): lhsT=(I,0)/(0,I) selects one chunk of an
    adjacent bias-chunk pair, so the bias stays resident in SBUF stored
    once (3.2KB/partition/head, loaded one time, no per-batch DMA).
  - v projection as fp8 hi/lo split (x = x8h + x8l, Wv = w8h + w8l) with
    three K=256 DoubleRow passes per psum tile, dropping the lo*lo term:
    ~2.7x fewer PE cycles than f32r at bf16-level accuracy. kT/q stay
    f32r: their quantization noise would amplify through exp by sqrt(d).
  - softmax sums: e-tiles accumulated on DVE (bf16 2x mode, in-place
    chain) + one ones-matmul per head instead of 10 PE ones-matmuls.
  - hswish on Pool/DVE: t = min(Relu(o+3+bv), 6) via Pool tensor_scalar
    ops, th = (o+bv)*t via Pool scalar_tensor_tensor; normalize mult on
    Pool; bv folds out of attn@v (softmax rows sum to 1).
  - psum->sbuf copies split across ACT/DVE (GPSIMD cannot touch PSUM on
    real hw); out-proj bias fused into the DVE psum->sbuf add.
  - software pipelining: batch b+1's kT/q/v projection psum tiles are
    emitted interleaved between batch b's attention heads (2-3 tiles per
    head), and the pair output projection interleaves with the following
    batch, keeping the PE fed through the shared psum-pool rotation.
  - PSUM: scores pool 3x[128,2,512] (chunk pairs at bank-aligned 512
    offsets, exp reads the [*, :320] pair in one ACT instr), po + psm
    1 bank each = 8 banks.
"""

import sys

if "/opt/trn_rl_repo" not in sys.path:
    sys.path.insert(0, "/opt/trn_rl_repo")

import ml_dtypes
import numpy as np

# --- problem constants (hardcoded, must match the grading reference) ---
B, N, C = 32, 1280, 256
H, KD, D = 8, 64, 128          # heads, key dim, value dim per head
NQ = 320                       # subsampled sequence length
OUT = 384
NCORES = 8
BPC = B // NCORES              # batches per core
EPS = 1e-5
NCH = N // 128                 # 10 n-chunks of 128
GRP = 2                        # scores psum group size (n-chunks per group)

# per-head engine tuning: bias add on PE (fp8 DoubleRow) vs DVE (exp-bias mult)
BIAS_PE = [True] * 8
# per-head: softmax sums via 10 PE ones-matmuls vs DVE accumulate + 1 matmul
SUMS_PE = [False] * 8

_PE_HEADS = [h for h in range(H) if BIAS_PE[h]]
_DVE_HEADS = [h for h in range(H) if not BIAS_PE[h]]
_PE_SLOT = {h: i for i, h in enumerate(_PE_HEADS)}
_DVE_SLOT = {h: i for i, h in enumerate(_DVE_HEADS)}

_SUB_IDX = np.concatenate([
    (np.arange(32)[::2][:, None] * 32 + np.arange(32)[::2][None, :]).reshape(-1),
    1024 + (np.arange(16)[::2][:, None] * 16 + np.arange(16)[::2][None, :]).reshape(-1),
])  # [320] subsample row gather


def _prep(inputs):
    """Host-side: fold BN into weights, reorder channels, shard over cores."""
    f32 = np.float32
    x = np.asarray(inputs["x"], f32)
    g_kv, b_kv = np.asarray(inputs["g_kv"], f32), np.asarray(inputs["b_kv"], f32)
    rm_kv, rv_kv = np.asarray(inputs["rm_kv"], f32), np.asarray(inputs["rv_kv"], f32)
    g_q, b_q = np.asarray(inputs["g_q"], f32), np.asarray(inputs["b_q"], f32)
    rm_q, rv_q = np.asarray(inputs["rm_q"], f32), np.asarray(inputs["rv_q"], f32)
    g_p, b_p = np.asarray(inputs["g_p"], f32), np.asarray(inputs["b_p"], f32)
    rm_p, rv_p = np.asarray(inputs["rm_p"], f32), np.asarray(inputs["rv_p"], f32)
    W_kv = np.asarray(inputs["W_kv"], f32)
    W_q = np.asarray(inputs["W_q"], f32)
    W_p = np.asarray(inputs["W_p"], f32)
    attn_bias = np.asarray(inputs["attn_bias"], f32)
    bias_idxs = np.asarray(inputs["bias_idxs"])

    s_kv = g_kv / np.sqrt(rv_kv + EPS)
    Wkv_f = W_kv * s_kv[:, None]
    bkv_f = b_kv - rm_kv * s_kv
    kidx = np.concatenate([np.arange(h * 192, h * 192 + KD) for h in range(H)])
    vidx = np.concatenate([np.arange(h * 192 + KD, (h + 1) * 192) for h in range(H)])
    wkt = np.ascontiguousarray(Wkv_f[kidx].T).reshape(2, 128, 512)     # [c,128][512 kch]
    wvt = np.ascontiguousarray(Wkv_f[vidx].T).reshape(2, 128, 1024)
    bvd = np.ascontiguousarray(bkv_f[vidx].reshape(8, 128).T)          # [128, H]

    scale = KD ** -0.5
    s_q = g_q / np.sqrt(rv_q + EPS)
    wqt = np.ascontiguousarray((W_q * (s_q * scale)[:, None]).T).reshape(2, 128, 512)
    bq = np.ascontiguousarray(((b_q - rm_q * s_q) * scale).reshape(4, 128).T)

    s_p = g_p / np.sqrt(rv_p + EPS)
    wpt = np.ascontiguousarray((W_p * s_p[:, None]).T / 6.0).reshape(
        8, 128, OUT).astype(ml_dtypes.bfloat16)
    bps = np.ascontiguousarray(np.broadcast_to(b_p - rm_p * s_p, (128, OUT))).astype(np.float32)

    biasT = attn_bias[:, bias_idxs].transpose(0, 2, 1)                 # [H, N, NQ]
    bias_cpq = biasT.reshape(H, NCH, 128, NQ).transpose(0, 2, 1, 3)    # [H,128,NCH,NQ]
    f8 = ml_dtypes.float8_e4m3
    # bias fp8, stored once per head; the DoubleRow identity pair (I,0)/(0,I)
    # selects one chunk of an adjacent pair per instruction
    bt8 = np.ascontiguousarray(bias_cpq).astype(f8)                    # [H,128,NCH,NQ]

    identp = np.zeros((128, 2, 2, 128), f8)
    identp[np.arange(128), 0, 0, np.arange(128)] = 1.0
    identp[np.arange(128), 1, 1, np.arange(128)] = 1.0

    wv8h = wvt.astype(f8)
    wv8l = (wvt - wv8h.astype(np.float32)).astype(f8)

    xs = x[:, _SUB_IDX, :]                                             # [B, NQ, C]
    in_maps = []
    for i in range(NCORES):
        sl = slice(i * BPC, (i + 1) * BPC)
        xt = np.ascontiguousarray(x[sl].transpose(0, 2, 1)).reshape(BPC, 2, 128, N)
        x8h = xt.astype(f8)
        x8l = (xt - x8h.astype(np.float32)).astype(f8)
        xst = np.ascontiguousarray(xs[sl].transpose(0, 2, 1)).reshape(BPC, 2, 128, NQ)
        in_maps.append({
            "xt": xt, "xst": xst, "x8h": x8h, "x8l": x8l,
            "wv8h": wv8h, "wv8l": wv8l,
            "wkt": wkt, "wvt": wvt, "wqt": wqt, "wpt": wpt,
            "bq": bq, "bv": bvd, "bv3": bvd + 3.0, "bps": bps,
            "bt8": bt8,
            "ones": np.ones((128, 128), ml_dtypes.bfloat16),
            "identp": identp,
        })
    return in_maps


def _body(tc, a, out_ap):
    import concourse.bass as bass  # noqa: F401
    import concourse.mybir as mybir
    from contextlib import ExitStack

    nc = tc.nc
    f32 = mybir.dt.float32
    f32r = mybir.dt.float32r
    bf16 = mybir.dt.bfloat16
    f8e4 = mybir.dt.float8e4
    AF = mybir.ActivationFunctionType
    ALU = mybir.AluOpType
    PM = mybir.MatmulPerfMode

    with ExitStack() as ctx:
        ctx.enter_context(
            nc.allow_low_precision(reason="bf16 o-side + fp8 bias matmuls are deliberate; verified vs fp32 reference")
        )
        singles = ctx.enter_context(tc.tile_pool(name="singles", bufs=1))
        # DMA order matters at startup: first-needed weights first (wk -> q/kT
        # projections of batch 0), small attention-phase tiles later.
        wk = singles.tile([128, 2, 512], f32r)
        nc.sync.dma_start(wk[:, :, 0:128], a["wkt"][:, :, 0:128].rearrange("c p j -> p c j"))
        wq = singles.tile([128, 2, 512], f32r)
        bqs = singles.tile([128, 4], f32)
        wv8h = singles.tile([128, 2, 1024], f8e4)
        wv8l = singles.tile([128, 2, 1024], f8e4)
        wp = singles.tile([128, 8, OUT], bf16)
        bvs = singles.tile([128, H], f32)
        bvs3 = singles.tile([128, H], f32)
        ones = singles.tile([128, 128], bf16)
        identp = singles.tile([128, 2, 2, 128], f8e4)
        bps = singles.tile([128, OUT], f32)
        bt8s = [singles.tile([128, NCH, NQ], f8e4, name=f"bt8h{h}")
                for h in range(H)]

        xt_p = ctx.enter_context(tc.tile_pool(name="xt", bufs=2))
        xst_p = ctx.enter_context(tc.tile_pool(name="xst", bufs=2))
        x8_p = ctx.enter_context(tc.tile_pool(name="x8", bufs=2))
        kt_p = ctx.enter_context(tc.tile_pool(name="kt", bufs=2))
        v_p = ctx.enter_context(tc.tile_pool(name="v", bufs=2))
        qt_p = ctx.enter_context(tc.tile_pool(name="qt", bufs=2))
        e_p = ctx.enter_context(tc.tile_pool(name="e", bufs=6))
        esum_p = ctx.enter_context(tc.tile_pool(name="esum", bufs=1))
        orw_p = ctx.enter_context(tc.tile_pool(name="orw", bufs=2))
        rc_p = ctx.enter_context(tc.tile_pool(name="rc", bufs=2))
        oh_p = ctx.enter_context(tc.tile_pool(name="oh", bufs=2))
        t1_p = ctx.enter_context(tc.tile_pool(name="t1", bufs=2))
        hs_p = ctx.enter_context(tc.tile_pool(name="hs", bufs=2))
        ob_p = ctx.enter_context(tc.tile_pool(name="ob", bufs=2))
        ps_sg = ctx.enter_context(tc.tile_pool(name="ps_sg", bufs=3, space="PSUM"))
        ps_o = ctx.enter_context(tc.tile_pool(name="ps_o", bufs=1, space="PSUM"))
        ps_sum = ctx.enter_context(tc.tile_pool(name="ps_sum", bufs=1, space="PSUM"))

        _wt_n = [0]

        def sg_tile():
            _wt_n[0] += 1
            return ps_sg.tile([128, GRP, 512], f32, tag="sg", name=f"sg{_wt_n[0]}")

        out_flat = out_ap.rearrange("b q o -> (b q) o")

        def dma_x(b, first=False):
            """Issue input DMAs for batch b; returns (xt, xst) tiles."""
            xt = xt_p.tile([128, 2, N], f32r, tag="xt", name=f"xt{b}")
            for ns in range(3):
                n0 = ns * 512
                nsz = min(512, N - n0)
                nc.sync.dma_start(
                    xt[:, :, n0:n0 + nsz],
                    a["xt"][b, :, :, n0:n0 + nsz].rearrange("c p n -> p c n"),
                )
                if first and ns == 0:
                    nc.sync.dma_start(
                        wk[:, :, 128:512],
                        a["wkt"][:, :, 128:512].rearrange("c p j -> p c j"))
                if first and ns == 1:
                    nc.sync.dma_start(wq, a["wqt"].rearrange("c p j -> p c j"))
                    nc.sync.dma_start(bqs, a["bq"])
            xst = xst_p.tile([128, 2, NQ], f32r, tag="xst", name=f"xst{b}")
            nc.sync.dma_start(xst, a["xst"][b].rearrange("c p n -> p c n"))
            x8h = x8_p.tile([128, 2, N], f8e4, tag="x8h", name=f"x8h{b}")
            nc.sync.dma_start(x8h, a["x8h"][b].rearrange("c p n -> p c n"))
            x8l = x8_p.tile([128, 2, N], f8e4, tag="x8l", name=f"x8l{b}")
            nc.sync.dma_start(x8l, a["x8l"][b].rearrange("c p n -> p c n"))
            return xt, xst, x8h, x8l

        def proj_gen(b, xt, xst, x8h, x8l):
            """Yield after each proj psum tile; returns (kt, vt, qt) eagerly."""
            kt = kt_p.tile([128, 4, N], f32r, tag="kt", name=f"kt{b}")
            vt = v_p.tile([128, NCH, 1024], bf16, tag="vt", name=f"vt{b}")
            qt = qt_p.tile([128, 4, NQ], f32r, tag="qt", name=f"qt{b}")

            def emit():
                # kT projection: no bias (softmax-invariant), wide copies
                for pr in range(4):
                    ps = sg_tile()
                    for half in range(2):       # n slices 0:512, 512:1024
                        n0 = half * 512
                        for cc in range(2):
                            nc.tensor.matmul(
                                ps[:, half, :],
                                lhsT=wk[:, cc, pr * 128:(pr + 1) * 128],
                                rhs=xt[:, cc, n0:n0 + 512],
                                start=(cc == 0), stop=(cc == 1),
                            )
                    if pr < 2:
                        nc.vector.tensor_copy(
                            kt[:, pr, 0:1024], ps.rearrange("p g j -> p (g j)"),
                        )
                    else:
                        nc.scalar.copy(
                            kt[:, pr, 0:1024], ps.rearrange("p g j -> p (g j)"),
                        )
                    yield
                ps = sg_tile()                  # 256-col tails, two prs per tile
                for prh in range(2):
                    for j in range(2):
                        pr = 2 * prh + j
                        for cc in range(2):
                            nc.tensor.matmul(
                                ps[:, j, :256] if prh == 0 else ps[:, j, 256:512],
                                lhsT=wk[:, cc, pr * 128:(pr + 1) * 128],
                                rhs=xt[:, cc, 1024:N],
                                start=(cc == 0), stop=(cc == 1),
                            )
                        if prh == 0:
                            nc.scalar.copy(kt[:, pr, 1024:N], ps[:, j, :256])
                        else:
                            nc.scalar.copy(kt[:, pr, 1024:N], ps[:, j, 256:512])
                yield
                # q projection: 2 prs per tile, fused bias on DVE
                for half in range(2):
                    ps = sg_tile()
                    for j in range(2):
                        pr = 2 * half + j
                        for cc in range(2):
                            nc.tensor.matmul(
                                ps[:, j, :NQ],
                                lhsT=wq[:, cc, pr * 128:(pr + 1) * 128],
                                rhs=xst[:, cc, :],
                                start=(cc == 0), stop=(cc == 1),
                            )
                    nc.vector.tensor_tensor(
                        qt[:, 2 * half:2 * half + 2, :], ps[:, :, :NQ],
                        bqs[:, 2 * half:2 * half + 2].to_broadcast((128, 2, NQ)),
                        ALU.add,
                    )
                    yield
                # v projection: fp8 hi/lo DoubleRow (K=256 per pass, 3 passes)
                for cn in range(NCH):
                    ps = sg_tile()
                    for hf in range(2):
                        for pi, (xx, ww) in enumerate(
                                ((x8h, wv8h), (x8l, wv8h), (x8h, wv8l))):
                            nc.tensor.matmul(
                                ps[:, hf, :],
                                lhsT=xx[:, :, cn * 128:(cn + 1) * 128],
                                rhs=ww[:, :, hf * 512:(hf + 1) * 512],
                                start=(pi == 0), stop=(pi == 2),
                                perf_mode=PM.DoubleRow,
                            )
                    if cn % 2 == 0:
                        nc.vector.tensor_copy(
                            vt[:, cn, :], ps.rearrange("p g j -> p (g j)"))
                    else:
                        nc.scalar.copy(
                            vt[:, cn, :], ps.rearrange("p g j -> p (g j)"))
                    yield

            return kt, vt, qt, emit()

        def attention(b, h, kt, vt, qt, t2):
            pr, p0 = h // 2, 64 * (h % 2)
            on_pe = BIAS_PE[h]
            bt8 = bt8s[h]
            po = ps_o.tile([128, NQ], f32, tag="po", name=f"po_{b}_{h}")
            e_tiles = []
            for g in range(NCH // GRP):
                sg = sg_tile()
                for j in range(GRP):
                    c = GRP * g + j
                    nc.tensor.matmul(
                        sg[:, j, :NQ],
                        lhsT=kt[p0:p0 + 64, pr, c * 128:(c + 1) * 128],
                        rhs=qt[p0:p0 + 64, pr, :],
                        start=True, stop=(not on_pe),
                    )
                    if on_pe:
                        nc.tensor.matmul(
                            sg[:, j, :NQ],
                            lhsT=identp[:, j, :, :],
                            rhs=bt8[:, GRP * g:GRP * (g + 1), :],
                            start=False, stop=True,
                            perf_mode=PM.DoubleRow,
                        )
                e = e_p.tile([128, GRP, NQ], bf16)
                nc.scalar.activation(e, sg[:, :, :NQ], AF.Exp)
                e_tiles.append(e)
                for j in range(GRP):
                    c = GRP * g + j
                    nc.tensor.matmul(
                        po[:, :NQ],
                        lhsT=vt[:, c, h * 128:(h + 1) * 128],
                        rhs=e[:, j, :],
                        start=(c == 0), stop=(c == NCH - 1),
                    )
            # softmax denominators
            psm = ps_sum.tile([128, NQ], f32, tag="psm", name=f"psm_{b}_{h}")
            if SUMS_PE[h]:
                for g in range(NCH // GRP):
                    for j in range(GRP):
                        c = GRP * g + j
                        nc.tensor.matmul(
                            psm,
                            lhsT=ones,
                            rhs=e_tiles[g][:, j, :],
                            start=(c == 0), stop=(c == NCH - 1),
                        )
            else:
                acc = e_tiles[1]
                nc.vector.tensor_tensor(acc, e_tiles[0], e_tiles[1], ALU.add)
                for g in range(2, NCH // GRP):
                    nc.vector.tensor_tensor(acc, acc, e_tiles[g], ALU.add)
                esum = esum_p.tile([128, NQ], bf16)
                nc.vector.tensor_tensor(esum, acc[:, 0, :], acc[:, 1, :], ALU.add)
                nc.tensor.matmul(psm, lhsT=ones, rhs=esum, start=True, stop=True)
            rc = rc_p.tile([128, NQ], bf16)
            nc.vector.reciprocal(rc, psm)
            oraw = orw_p.tile([128, NQ], bf16, tag="oraw", name=f"oraw_{b}_{h}")
            nc.vector.tensor_copy(oraw, po)
            oh = oh_p.tile([128, NQ], bf16)
            nc.gpsimd.tensor_tensor(oh, oraw, rc, ALU.mult)
            # hswish: t = min(Relu(o + 3 + bv), 6);  th = (o + bv) * t
            t1 = t1_p.tile([128, NQ], bf16)
            nc.gpsimd.tensor_scalar(t1, oh, bvs3[:, h:h + 1], 0.0, ALU.add, ALU.max)
            nc.gpsimd.tensor_scalar(t1, t1, 6.0, None, ALU.min)
            obv = t1_p.tile([128, NQ], bf16, tag="obv", name=f"obv_{b}_{h}")
            nc.gpsimd.tensor_scalar(obv, oh, bvs[:, h:h + 1], None, ALU.add)
            nc.gpsimd.tensor_tensor(t2[:, h, b % 2, :], obv, t1, ALU.mult)

        def out_proj(b, t2, qcs=range(5)):
            for qc in qcs:
                r0 = (b - 1) * NQ + qc * 128
                ps = sg_tile()
                for dc in range(8):
                    nc.tensor.matmul(
                        ps[:, 0, :OUT],
                        lhsT=t2[:, dc, :, :].rearrange(
                            "p bb q -> p (bb q)")[:, qc * 128:(qc + 1) * 128],
                        rhs=wp[:, dc, :],
                        start=(dc == 0), stop=(dc == 7),
                    )
                ob = ob_p.tile([128, OUT], f32)
                nc.vector.tensor_tensor(ob, ps[:, 0, :OUT], bps, ALU.add)
                nc.sync.dma_start(out_flat[r0:r0 + 128, :], ob)
                yield

        # prologue: batch 0 inputs + weights; proj(0) up to attention-ready
        xt0, xst0, x8h0, x8l0 = dma_x(0, first=True)
        nc.sync.dma_start(wv8h, a["wv8h"].rearrange("c p j -> p c j"))
        nc.sync.dma_start(wv8l, a["wv8l"].rearrange("c p j -> p c j"))
        nc.sync.dma_start(identp, a["identp"])
        nc.sync.dma_start(bt8s[0], a["bt8"][0])
        nc.sync.dma_start(bt8s[1], a["bt8"][1])
        nc.sync.dma_start(ones, a["ones"])
        nc.sync.dma_start(bvs, a["bv"])
        nc.sync.dma_start(bvs3, a["bv3"])
        for h in range(2, H):
            nc.sync.dma_start(bt8s[h], a["bt8"][h])
        nc.sync.dma_start(wp, a["wpt"].rearrange("c p j -> p c j"))
        nc.sync.dma_start(bps, a["bps"])

        kt, vt, qt, gen0 = proj_gen(0, xt0, xst0, x8h0, x8l0)
        for _ in gen0:          # batch 0 proj must fully precede its attention
            pass
        pending = []
        t2 = None
        nxt = None
        for b in range(BPC):
            if b % 2 == 0:
                t2 = hs_p.tile([128, H, 2, NQ], bf16, tag="t2", name=f"t2_{b}")
            # interleave remaining proj tiles (this batch's tail + next batch)
            if b + 1 < BPC:
                xtn, xstn, x8hn, x8ln = dma_x(b + 1)
                nxt = proj_gen(b + 1, xtn, xstn, x8hn, x8ln)
                pending.append(nxt[3])
            for h in range(H):
                attention(b, h, kt, vt, qt, t2)
                for _ in range(3 if h >= 5 else 2):
                    while pending:
                        if next(pending[0], "done") == "done":
                            pending.pop(0)
                        else:
                            break
            while pending:
                if next(pending[0], "done") == "done":
                    pending.pop(0)
                else:
                    break
            if pending:
                for _ in pending[0]:
                    pass
                pending.pop(0)
            if b % 2 == 1:
                if b + 1 < BPC:
                    pending.append(out_proj(b, t2))  # interleave with next batch
                else:
                    for _ in out_proj(b, t2):
                        pass
            if nxt is not None:
                kt, vt, qt = nxt[0], nxt[1], nxt[2]
                nxt = None


def build():
    import concourse.mybir as mybir
    import concourse.tile as tile
    from concourse import bacc

    nc = bacc.Bacc("TRN2", target_bir_lowering=False, debug=False)
    f32, bf16 = mybir.dt.float32, mybir.dt.bfloat16
    f8e4 = mybir.dt.float8e4
    a = {}

    def din(name, shape, dt=f32):
        a[name] = nc.dram_tensor(name, shape, dt, kind="ExternalInput").ap()

    f32r = mybir.dt.float32r
    din("xt", [BPC, 2, 128, N], f32r)
    din("xst", [BPC, 2, 128, NQ], f32r)
    din("wkt", [2, 128, 512], f32r)
    din("wvt", [2, 128, 1024], f32r)
    din("x8h", [BPC, 2, 128, N], f8e4)
    din("x8l", [BPC, 2, 128, N], f8e4)
    din("wv8h", [2, 128, 1024], f8e4)
    din("wv8l", [2, 128, 1024], f8e4)
    din("wqt", [2, 128, 512], f32r)
    din("wpt", [8, 128, OUT], bf16)
    din("bq", [128, 4])
    din("bv", [128, H])
    din("bv3", [128, H])
    din("bps", [128, OUT])
    din("bt8", [H, 128, NCH, NQ], f8e4)
    din("ones", [128, 128], bf16)
    din("identp", [128, 2, 2, 128], f8e4)
    out_ap = nc.dram_tensor("out", [BPC, NQ, OUT], f32, kind="ExternalOutput").ap()

    with tile.TileContext(nc) as tc:
        _body(tc, a, out_ap)
    nc.compile()
    return nc


_NC_CACHE = None


def _get_nc():
    global _NC_CACHE
    if _NC_CACHE is None:
        _NC_CACHE = build()
    return _NC_CACHE


def kernel(**inputs):
    from concourse.bass_utils import run_bass_kernel_spmd

    in_maps = _prep(inputs)
    nc = _get_nc()
    res = run_bass_kernel_spmd(nc, in_maps, list(range(NCORES)))
    out = np.concatenate([res.results[i]["out"] for i in range(NCORES)], axis=0)
    return np.ascontiguousarray(out, dtype=np.float32)


if __name__ == "__main__":
    rng = np.random.default_rng(0)
    print("smoke: building bass module...")
    nc = build()
    print("built ok:", sum(len(bb.instructions) for bb in nc.m.functions[0].blocks), "instructions")


# revision 7
# speedup vs baseline: 1.2974x; 1.0320x over previous
"""AttentionSubsample Trainium2 kernel.

Full (unsharded) inputs in, full output out. Data-parallel over batch:
32 batches -> 8 NeuronCores x 4 batches each. Weights/biases replicated.

Engine-balance design (cost-model 212.4us/core, vs 275.6us v1 baseline):
  - k-channel BN bias dropped entirely: softmax over n is invariant to
    per-q shifts and (k+bk)@q shifts every key n equally.
  - score bias added pre-exp on the PE as fp8(e4m3) DoubleRow identity
    matmuls (0.5 cyc/row): lhsT=(I,0)/(0,I) selects one chunk of an
    adjacent bias-chunk pair, so the bias stays resident in SBUF stored
    once (3.2KB/partition/head, loaded one time, no per-batch DMA).
  - v projection as fp8 hi/lo split (x = x8h + x8l, Wv = w8h + w8l) with
    three K=256 DoubleRow passes per psum tile, dropping the lo*lo term:
    ~2.7x fewer PE cycles than f32r at bf16-level accuracy. kT/q stay
    f32r: their quantization noise would amplify through exp by sqrt(d).
  - softmax sums: e-tiles accumulated on DVE (bf16 2x mode, in-place
    chain) + one ones-matmul per head instead of 10 PE ones-matmuls.
  - hswish on Pool/DVE: t = min(Relu(o+3+bv), 6) via Pool tensor_scalar
    ops, th = (o+bv)*t via Pool scalar_tensor_tensor; normalize mult on
    Pool; bv folds out of attn@v (softmax rows sum to 1).
  - psum->sbuf copies split across ACT/DVE (GPSIMD cannot touch PSUM on
    real hw); out-proj bias fused into the DVE psum->sbuf add.
  - software pipelining: batch b+1's kT/q/v projection psum tiles are
    emitted interleaved between batch b's attention score groups (one
    tile after every group, via generators), and the pair output
    projection interleaves with the following batch, keeping the PE fed
    through the shared psum-pool rotation.
  - PSUM: scores pool 3x[128,2,512] (chunk pairs at bank-aligned 512
    offsets, exp reads the [*, :320] pair in one ACT instr), po + psm
    1 bank each = 8 banks.
"""

import sys

if "/opt/trn_rl_repo" not in sys.path:
    sys.path.insert(0, "/opt/trn_rl_repo")

import ml_dtypes
import numpy as np

# --- problem constants (hardcoded, must match the grading reference) ---
B, N, C = 32, 1280, 256
H, KD, D = 8, 64, 128          # heads, key dim, value dim per head
NQ = 320                       # subsampled sequence length
OUT = 384
NCORES = 8
BPC = B // NCORES              # batches per core
EPS = 1e-5
NCH = N // 128                 # 10 n-chunks of 128
GRP = 2                        # scores psum group size (n-chunks per group)

# per-head engine tuning: bias add on PE (fp8 DoubleRow) vs DVE (exp-bias mult)
BIAS_PE = [True] * 8
# per-head: softmax sums via 10 PE ones-matmuls vs DVE accumulate + 1 matmul
SUMS_PE = [False] * 8

_PE_HEADS = [h for h in range(H) if BIAS_PE[h]]
_DVE_HEADS = [h for h in range(H) if not BIAS_PE[h]]
_PE_SLOT = {h: i for i, h in enumerate(_PE_HEADS)}
_DVE_SLOT = {h: i for i, h in enumerate(_DVE_HEADS)}

_SUB_IDX = np.concatenate([
    (np.arange(32)[::2][:, None] * 32 + np.arange(32)[::2][None, :]).reshape(-1),
    1024 + (np.arange(16)[::2][:, None] * 16 + np.arange(16)[::2][None, :]).reshape(-1),
])  # [320] subsample row gather


def _prep(inputs):
    """Host-side: fold BN into weights, reorder channels, shard over cores."""
    f32 = np.float32
    x = np.asarray(inputs["x"], f32)
    g_kv, b_kv = np.asarray(inputs["g_kv"], f32), np.asarray(inputs["b_kv"], f32)
    rm_kv, rv_kv = np.asarray(inputs["rm_kv"], f32), np.asarray(inputs["rv_kv"], f32)
    g_q, b_q = np.asarray(inputs["g_q"], f32), np.asarray(inputs["b_q"], f32)
    rm_q, rv_q = np.asarray(inputs["rm_q"], f32), np.asarray(inputs["rv_q"], f32)
    g_p, b_p = np.asarray(inputs["g_p"], f32), np.asarray(inputs["b_p"], f32)
    rm_p, rv_p = np.asarray(inputs["rm_p"], f32), np.asarray(inputs["rv_p"], f32)
    W_kv = np.asarray(inputs["W_kv"], f32)
    W_q = np.asarray(inputs["W_q"], f32)
    W_p = np.asarray(inputs["W_p"], f32)
    attn_bias = np.asarray(inputs["attn_bias"], f32)
    bias_idxs = np.asarray(inputs["bias_idxs"])

    s_kv = g_kv / np.sqrt(rv_kv + EPS)
    Wkv_f = W_kv * s_kv[:, None]
    bkv_f = b_kv - rm_kv * s_kv
    kidx = np.concatenate([np.arange(h * 192, h * 192 + KD) for h in range(H)])
    vidx = np.concatenate([np.arange(h * 192 + KD, (h + 1) * 192) for h in range(H)])
    wkt = np.ascontiguousarray(Wkv_f[kidx].T).reshape(2, 128, 512)     # [c,128][512 kch]
    wvt = np.ascontiguousarray(Wkv_f[vidx].T).reshape(2, 128, 1024)
    bvd = np.ascontiguousarray(bkv_f[vidx].reshape(8, 128).T)          # [128, H]

    scale = KD ** -0.5
    s_q = g_q / np.sqrt(rv_q + EPS)
    wqt = np.ascontiguousarray((W_q * (s_q * scale)[:, None]).T).reshape(2, 128, 512)
    bq = np.ascontiguousarray(((b_q - rm_q * s_q) * scale).reshape(4, 128).T)

    s_p = g_p / np.sqrt(rv_p + EPS)
    wpt = np.ascontiguousarray((W_p * s_p[:, None]).T / 6.0).reshape(
        8, 128, OUT).astype(ml_dtypes.bfloat16)
    bps = np.ascontiguousarray(np.broadcast_to(b_p - rm_p * s_p, (128, OUT))).astype(np.float32)

    biasT = attn_bias[:, bias_idxs].transpose(0, 2, 1)                 # [H, N, NQ]
    bias_cpq = biasT.reshape(H, NCH, 128, NQ).transpose(0, 2, 1, 3)    # [H,128,NCH,NQ]
    f8 = ml_dtypes.float8_e4m3
    # bias fp8, stored once per head; the DoubleRow identity pair (I,0)/(0,I)
    # selects one chunk of an adjacent pair per instruction
    bt8 = np.ascontiguousarray(bias_cpq).astype(f8)                    # [H,128,NCH,NQ]

    identp = np.zeros((128, 2, 2, 128), f8)
    identp[np.arange(128), 0, 0, np.arange(128)] = 1.0
    identp[np.arange(128), 1, 1, np.arange(128)] = 1.0

    wv8h = wvt.astype(f8)
    wv8l = (wvt - wv8h.astype(np.float32)).astype(f8)

    xs = x[:, _SUB_IDX, :]                                             # [B, NQ, C]
    in_maps = []
    for i in range(NCORES):
        sl = slice(i * BPC, (i + 1) * BPC)
        xt = np.ascontiguousarray(x[sl].transpose(0, 2, 1)).reshape(BPC, 2, 128, N)
        x8h = xt.astype(f8)
        x8l = (xt - x8h.astype(np.float32)).astype(f8)
        xst = np.ascontiguousarray(xs[sl].transpose(0, 2, 1)).reshape(BPC, 2, 128, NQ)
        in_maps.append({
            "xt": xt, "xst": xst, "x8h": x8h, "x8l": x8l,
            "wv8h": wv8h, "wv8l": wv8l,
            "wkt": wkt, "wvt": wvt, "wqt": wqt, "wpt": wpt,
            "bq": bq, "bv": bvd, "bv3": bvd + 3.0, "bps": bps,
            "bt8": bt8,
            "ones": np.ones((128, 128), ml_dtypes.bfloat16),
            "identp": identp,
        })
    return in_maps


def _body(tc, a, out_ap):
    import concourse.bass as bass  # noqa: F401
    import concourse.mybir as mybir
    from contextlib import ExitStack

    nc = tc.nc
    f32 = mybir.dt.float32
    f32r = mybir.dt.float32r
    bf16 = mybir.dt.bfloat16
    f8e4 = mybir.dt.float8e4
    AF = mybir.ActivationFunctionType
    ALU = mybir.AluOpType
    PM = mybir.MatmulPerfMode

    with ExitStack() as ctx:
        ctx.enter_context(
            nc.allow_low_precision(reason="bf16 o-side + fp8 bias matmuls are deliberate; verified vs fp32 reference")
        )
        singles = ctx.enter_context(tc.tile_pool(name="singles", bufs=1))
        # DMA order matters at startup: first-needed weights first (wk -> q/kT
        # projections of batch 0), small attention-phase tiles later.
        wk = singles.tile([128, 2, 512], f32r)
        nc.sync.dma_start(wk[:, :, 0:128], a["wkt"][:, :, 0:128].rearrange("c p j -> p c j"))
        wq = singles.tile([128, 2, 512], f32r)
        bqs = singles.tile([128, 4], f32)
        wv8h = singles.tile([128, 2, 1024], f8e4)
        wv8l = singles.tile([128, 2, 1024], f8e4)
        wp = singles.tile([128, 8, OUT], bf16)
        bvs = singles.tile([128, H], f32)
        bvs3 = singles.tile([128, H], f32)
        ones = singles.tile([128, 128], bf16)
        identp = singles.tile([128, 2, 2, 128], f8e4)
        bps = singles.tile([128, OUT], f32)
        bt8s = [singles.tile([128, NCH, NQ], f8e4, name=f"bt8h{h}")
                for h in range(H)]

        xt_p = ctx.enter_context(tc.tile_pool(name="xt", bufs=2))
        xst_p = ctx.enter_context(tc.tile_pool(name="xst", bufs=2))
        x8_p = ctx.enter_context(tc.tile_pool(name="x8", bufs=2))
        kt_p = ctx.enter_context(tc.tile_pool(name="kt", bufs=2))
        v_p = ctx.enter_context(tc.tile_pool(name="v", bufs=2))
        qt_p = ctx.enter_context(tc.tile_pool(name="qt", bufs=2))
        e_p = ctx.enter_context(tc.tile_pool(name="e", bufs=6))
        esum_p = ctx.enter_context(tc.tile_pool(name="esum", bufs=1))
        orw_p = ctx.enter_context(tc.tile_pool(name="orw", bufs=2))
        rc_p = ctx.enter_context(tc.tile_pool(name="rc", bufs=2))
        oh_p = ctx.enter_context(tc.tile_pool(name="oh", bufs=2))
        t1_p = ctx.enter_context(tc.tile_pool(name="t1", bufs=2))
        hs_p = ctx.enter_context(tc.tile_pool(name="hs", bufs=2))
        ob_p = ctx.enter_context(tc.tile_pool(name="ob", bufs=2))
        ps_sg = ctx.enter_context(tc.tile_pool(name="ps_sg", bufs=3, space="PSUM"))
        ps_o = ctx.enter_context(tc.tile_pool(name="ps_o", bufs=1, space="PSUM"))
        ps_sum = ctx.enter_context(tc.tile_pool(name="ps_sum", bufs=1, space="PSUM"))

        _wt_n = [0]

        def sg_tile():
            _wt_n[0] += 1
            return ps_sg.tile([128, GRP, 512], f32, tag="sg", name=f"sg{_wt_n[0]}")

        out_flat = out_ap.rearrange("b q o -> (b q) o")

        def dma_x(b, first=False, stagger=False):
            """Issue input DMAs for batch b; returns (xt, xst) tiles."""
            xt = xt_p.tile([128, 2, N], f32r, tag="xt", name=f"xt{b}")
            for ns in range(3):
                if stagger:        # keep the resident-bias stream fed first
                    nc.sync.dma_start(bt8s[2 + ns], a["bt8"][2 + ns])
                n0 = ns * 512
                nsz = min(512, N - n0)
                nc.sync.dma_start(
                    xt[:, :, n0:n0 + nsz],
                    a["xt"][b, :, :, n0:n0 + nsz].rearrange("c p n -> p c n"),
                )
                if first and ns == 0:
                    nc.sync.dma_start(
                        wk[:, :, 128:512],
                        a["wkt"][:, :, 128:512].rearrange("c p j -> p c j"))
                if first and ns == 1:
                    nc.sync.dma_start(wq, a["wqt"].rearrange("c p j -> p c j"))
                    nc.sync.dma_start(bqs, a["bq"])
            if stagger:
                nc.sync.dma_start(bt8s[5], a["bt8"][5])
            xst = xst_p.tile([128, 2, NQ], f32r, tag="xst", name=f"xst{b}")
            nc.sync.dma_start(xst, a["xst"][b].rearrange("c p n -> p c n"))
            x8h = x8_p.tile([128, 2, N], f8e4, tag="x8h", name=f"x8h{b}")
            nc.sync.dma_start(x8h, a["x8h"][b].rearrange("c p n -> p c n"))
            if stagger:
                nc.sync.dma_start(bt8s[6], a["bt8"][6])
            x8l = x8_p.tile([128, 2, N], f8e4, tag="x8l", name=f"x8l{b}")
            nc.sync.dma_start(x8l, a["x8l"][b].rearrange("c p n -> p c n"))
            if stagger:
                nc.sync.dma_start(bt8s[7], a["bt8"][7])
                nc.sync.dma_start(wp, a["wpt"].rearrange("c p j -> p c j"))
                nc.sync.dma_start(bps, a["bps"])
            return xt, xst, x8h, x8l

        def proj_gen(b, xt, xst, x8h, x8l):
            """Yield after each proj psum tile; returns (kt, vt, qt) eagerly."""
            kt = kt_p.tile([128, 4, N], f32r, tag="kt", name=f"kt{b}")
            vt = v_p.tile([128, NCH, 1024], bf16, tag="vt", name=f"vt{b}")
            qt = qt_p.tile([128, 4, NQ], f32r, tag="qt", name=f"qt{b}")

            def emit():
                # kT projection: no bias (softmax-invariant), wide copies
                for pr in range(4):
                    ps = sg_tile()
                    for half in range(2):       # n slices 0:512, 512:1024
                        n0 = half * 512
                        for cc in range(2):
                            nc.tensor.matmul(
                                ps[:, half, :],
                                lhsT=wk[:, cc, pr * 128:(pr + 1) * 128],
                                rhs=xt[:, cc, n0:n0 + 512],
                                start=(cc == 0), stop=(cc == 1),
                            )
                    if pr < 2:
                        nc.vector.tensor_copy(
                            kt[:, pr, 0:1024], ps.rearrange("p g j -> p (g j)"),
                        )
                    else:
                        nc.scalar.copy(
                            kt[:, pr, 0:1024], ps.rearrange("p g j -> p (g j)"),
                        )
                    yield
                ps = sg_tile()                  # 256-col tails, two prs per tile
                for prh in range(2):
                    for j in range(2):
                        pr = 2 * prh + j
                        for cc in range(2):
                            nc.tensor.matmul(
                                ps[:, j, :256] if prh == 0 else ps[:, j, 256:512],
                                lhsT=wk[:, cc, pr * 128:(pr + 1) * 128],
                                rhs=xt[:, cc, 1024:N],
                                start=(cc == 0), stop=(cc == 1),
                            )
                        if prh == 0:
                            nc.scalar.copy(kt[:, pr, 1024:N], ps[:, j, :256])
                        else:
                            nc.scalar.copy(kt[:, pr, 1024:N], ps[:, j, 256:512])
                yield
                # q projection: 2 prs per tile, fused bias on DVE
                for half in range(2):
                    ps = sg_tile()
                    for j in range(2):
                        pr = 2 * half + j
                        for cc in range(2):
                            nc.tensor.matmul(
                                ps[:, j, :NQ],
                                lhsT=wq[:, cc, pr * 128:(pr + 1) * 128],
                                rhs=xst[:, cc, :],
                                start=(cc == 0), stop=(cc == 1),
                            )
                    nc.vector.tensor_tensor(
                        qt[:, 2 * half:2 * half + 2, :], ps[:, :, :NQ],
                        bqs[:, 2 * half:2 * half + 2].to_broadcast((128, 2, NQ)),
                        ALU.add,
                    )
                    yield
                # v projection: fp8 hi/lo DoubleRow (K=256 per pass, 3 passes)
                for cn in range(NCH):
                    ps = sg_tile()
                    for hf in range(2):
                        for pi, (xx, ww) in enumerate(
                                ((x8h, wv8h), (x8l, wv8h), (x8h, wv8l))):
                            nc.tensor.matmul(
                                ps[:, hf, :],
                                lhsT=xx[:, :, cn * 128:(cn + 1) * 128],
                                rhs=ww[:, :, hf * 512:(hf + 1) * 512],
                                start=(pi == 0), stop=(pi == 2),
                                perf_mode=PM.DoubleRow,
                            )
                    if cn % 2 == 0:
                        nc.vector.tensor_copy(
                            vt[:, cn, :], ps.rearrange("p g j -> p (g j)"))
                    else:
                        nc.scalar.copy(
                            vt[:, cn, :], ps.rearrange("p g j -> p (g j)"))
                    yield

            return kt, vt, qt, emit()

        def attention(b, h, kt, vt, qt, t2):
            # generator: yields after each score group so the driver can
            # weave projection tiles of the next batch between groups
            yield
            pr, p0 = h // 2, 64 * (h % 2)
            on_pe = BIAS_PE[h]
            bt8 = bt8s[h]
            po = ps_o.tile([128, NQ], f32, tag="po", name=f"po_{b}_{h}")
            e_tiles = []
            for g in range(NCH // GRP):
                sg = sg_tile()
                for j in range(GRP):
                    c = GRP * g + j
                    nc.tensor.matmul(
                        sg[:, j, :NQ],
                        lhsT=kt[p0:p0 + 64, pr, c * 128:(c + 1) * 128],
                        rhs=qt[p0:p0 + 64, pr, :],
                        start=True, stop=(not on_pe),
                    )
                    if on_pe:
                        nc.tensor.matmul(
                            sg[:, j, :NQ],
                            lhsT=identp[:, j, :, :],
                            rhs=bt8[:, GRP * g:GRP * (g + 1), :],
                            start=False, stop=True,
                            perf_mode=PM.DoubleRow,
                        )
                e = e_p.tile([128, GRP, NQ], bf16)
                nc.scalar.activation(e, sg[:, :, :NQ], AF.Exp)
                e_tiles.append(e)
                for j in range(GRP):
                    c = GRP * g + j
                    nc.tensor.matmul(
                        po[:, :NQ],
                        lhsT=vt[:, c, h * 128:(h + 1) * 128],
                        rhs=e[:, j, :],
                        start=(c == 0), stop=(c == NCH - 1),
                    )
                yield
            # softmax denominators
            psm = ps_sum.tile([128, NQ], f32, tag="psm", name=f"psm_{b}_{h}")
            if SUMS_PE[h]:
                for g in range(NCH // GRP):
                    for j in range(GRP):
                        c = GRP * g + j
                        nc.tensor.matmul(
                            psm,
                            lhsT=ones,
                            rhs=e_tiles[g][:, j, :],
                            start=(c == 0), stop=(c == NCH - 1),
                        )
            else:
                acc = e_tiles[1]
                nc.vector.tensor_tensor(acc, e_tiles[0], e_tiles[1], ALU.add)
                for g in range(2, NCH // GRP):
                    nc.vector.tensor_tensor(acc, acc, e_tiles[g], ALU.add)
                esum = esum_p.tile([128, NQ], bf16)
                nc.vector.tensor_tensor(esum, acc[:, 0, :], acc[:, 1, :], ALU.add)
                nc.tensor.matmul(psm, lhsT=ones, rhs=esum, start=True, stop=True)
            rc = rc_p.tile([128, NQ], bf16)
            nc.vector.reciprocal(rc, psm)
            oraw = orw_p.tile([128, NQ], bf16, tag="oraw", name=f"oraw_{b}_{h}")
            nc.vector.tensor_copy(oraw, po)
            oh = oh_p.tile([128, NQ], bf16)
            nc.gpsimd.tensor_tensor(oh, oraw, rc, ALU.mult)
            # hswish: t = min(Relu(o + 3 + bv), 6);  th = (o + bv) * t
            t1 = t1_p.tile([128, NQ], bf16)
            nc.gpsimd.tensor_scalar(t1, oh, bvs3[:, h:h + 1], 0.0, ALU.add, ALU.max)
            nc.gpsimd.tensor_scalar(t1, t1, 6.0, None, ALU.min)
            obv = t1_p.tile([128, NQ], bf16, tag="obv", name=f"obv_{b}_{h}")
            nc.gpsimd.tensor_scalar(obv, oh, bvs[:, h:h + 1], None, ALU.add)
            nc.gpsimd.tensor_tensor(t2[:, h, b % 2, :], obv, t1, ALU.mult)

        def out_proj(b, t2, qcs=range(5)):
            for qc in qcs:
                r0 = (b - 1) * NQ + qc * 128
                ps = sg_tile()
                for dc in range(8):
                    nc.tensor.matmul(
                        ps[:, 0, :OUT],
                        lhsT=t2[:, dc, :, :].rearrange(
                            "p bb q -> p (bb q)")[:, qc * 128:(qc + 1) * 128],
                        rhs=wp[:, dc, :],
                        start=(dc == 0), stop=(dc == 7),
                    )
                ob = ob_p.tile([128, OUT], f32)
                nc.vector.tensor_tensor(ob, ps[:, 0, :OUT], bps, ALU.add)
                nc.sync.dma_start(out_flat[r0:r0 + 128, :], ob)
                yield

        # prologue: batch 0 inputs + weights; proj(0) up to attention-ready
        xt0, xst0, x8h0, x8l0 = dma_x(0, first=True)
        nc.sync.dma_start(identp, a["identp"])
        nc.sync.dma_start(wv8h, a["wv8h"].rearrange("c p j -> p c j"))
        nc.sync.dma_start(bt8s[0], a["bt8"][0])
        nc.sync.dma_start(wv8l, a["wv8l"].rearrange("c p j -> p c j"))
        nc.sync.dma_start(bt8s[1], a["bt8"][1])
        nc.sync.dma_start(ones, a["ones"])
        nc.sync.dma_start(bvs, a["bv"])
        nc.sync.dma_start(bvs3, a["bv3"])

        kt, vt, qt, gen0 = proj_gen(0, xt0, xst0, x8h0, x8l0)
        for _ in gen0:          # batch 0 proj must fully precede its attention
            pass
        pending = []
        t2 = None
        nxt = None
        for b in range(BPC):
            if b % 2 == 0:
                t2 = hs_p.tile([128, H, 2, NQ], bf16, tag="t2", name=f"t2_{b}")
            # interleave remaining proj tiles (this batch's tail + next batch)
            if b + 1 < BPC:
                xtn, xstn, x8hn, x8ln = dma_x(b + 1, stagger=(b == 0))
                nxt = proj_gen(b + 1, xtn, xstn, x8hn, x8ln)
                pending.append(nxt[3])
            for h in range(H):
                for gi, _ in enumerate(attention(b, h, kt, vt, qt, t2)):
                    if gi in (1, 2, 3, 4, 5):
                        while pending:
                            if next(pending[0], "done") == "done":
                                pending.pop(0)
                            else:
                                break
            while pending:
                if next(pending[0], "done") == "done":
                    pending.pop(0)
                else:
                    break
            if pending:
                for _ in pending[0]:
                    pass
                pending.pop(0)
            if b % 2 == 1:
                if b + 1 < BPC:
                    pending.append(out_proj(b, t2))  # interleave with next batch
                else:
                    for _ in out_proj(b, t2):
                        pass
            if nxt is not None:
                kt, vt, qt = nxt[0], nxt[1], nxt[2]
                nxt = None


def build():
    import concourse.mybir as mybir
    import concourse.tile as tile
    from concourse import bacc

    nc = bacc.Bacc("TRN2", target_bir_lowering=False, debug=False)
    f32, bf16 = mybir.dt.float32, mybir.dt.bfloat16
    f8e4 = mybir.dt.float8e4
    a = {}

    def din(name, shape, dt=f32):
        a[name] = nc.dram_tensor(name, shape, dt, kind="ExternalInput").ap()

    f32r = mybir.dt.float32r
    din("xt", [BPC, 2, 128, N], f32r)
    din("xst", [BPC, 2, 128, NQ], f32r)
    din("wkt", [2, 128, 512], f32r)
    din("wvt", [2, 128, 1024], f32r)
    din("x8h", [BPC, 2, 128, N], f8e4)
    din("x8l", [BPC, 2, 128, N], f8e4)
    din("wv8h", [2, 128, 1024], f8e4)
    din("wv8l", [2, 128, 1024], f8e4)
    din("wqt", [2, 128, 512], f32r)
    din("wpt", [8, 128, OUT], bf16)
    din("bq", [128, 4])
    din("bv", [128, H])
    din("bv3", [128, H])
    din("bps", [128, OUT])
    din("bt8", [H, 128, NCH, NQ], f8e4)
    din("ones", [128, 128], bf16)
    din("identp", [128, 2, 2, 128], f8e4)
    out_ap = nc.dram_tensor("out", [BPC, NQ, OUT], f32, kind="ExternalOutput").ap()

    with tile.TileContext(nc) as tc:
        _body(tc, a, out_ap)
    nc.compile()
    return nc


_NC_CACHE = None


def _get_nc():
    global _NC_CACHE
    if _NC_CACHE is None:
        _NC_CACHE = build()
    return _NC_CACHE


def kernel(**inputs):
    from concourse.bass_utils import run_bass_kernel_spmd

    in_maps = _prep(inputs)
    nc = _get_nc()
    res = run_bass_kernel_spmd(nc, in_maps, list(range(NCORES)))
    out = np.concatenate([res.results[i]["out"] for i in range(NCORES)], axis=0)
    return np.ascontiguousarray(out, dtype=np.float32)


if __name__ == "__main__":
    rng = np.random.default_rng(0)
    print("smoke: building bass module...")
    nc = build()
    print("built ok:", sum(len(bb.instructions) for bb in nc.m.functions[0].blocks), "instructions")


# revision 8
# speedup vs baseline: 1.3187x; 1.0165x over previous
"""AttentionSubsample Trainium2 kernel.

Full (unsharded) inputs in, full output out. Data-parallel over batch:
32 batches -> 8 NeuronCores x 4 batches each. Weights/biases replicated.

Engine-balance design (cost-model 209.0us/core, vs 275.6us v1 baseline):
  - k-channel BN bias dropped entirely: softmax over n is invariant to
    per-q shifts and (k+bk)@q shifts every key n equally.
  - score bias added pre-exp on the PE as fp8(e4m3) DoubleRow identity
    matmuls (0.5 cyc/row): lhsT=(I,0)/(0,I) selects one chunk of an
    adjacent bias-chunk pair, so the bias stays resident in SBUF stored
    once (3.2KB/partition/head, loaded one time, no per-batch DMA).
  - v projection as fp8 hi/lo split (x = x8h + x8l, Wv = w8h + w8l) with
    three K=256 DoubleRow passes per psum tile, dropping the lo*lo term:
    ~2.7x fewer PE cycles than f32r at bf16-level accuracy. kT/q stay
    f32r: their quantization noise would amplify through exp by sqrt(d).
  - softmax sums: e-tiles accumulated on DVE (bf16 2x mode, in-place
    chain) + one ones-matmul per head instead of 10 PE ones-matmuls.
  - hswish on Pool/DVE: t = min(Relu(o+3+bv), 6) via Pool tensor_scalar
    ops, th = (o+bv)*t via Pool scalar_tensor_tensor; normalize mult on
    Pool; bv folds out of attn@v (softmax rows sum to 1).
  - psum->sbuf copies split across ACT/DVE (GPSIMD cannot touch PSUM on
    real hw); out-proj bias fused into the DVE psum->sbuf add.
  - software pipelining: batch b+1's kT/q/v projection psum tiles are
    emitted interleaved between batch b's attention score groups (one
    tile after every group, via generators), and the pair output
    projection interleaves with the following batch, keeping the PE fed
    through the shared psum-pool rotation.
  - PSUM: scores pool 3x[128,2,512] (chunk pairs at bank-aligned 512
    offsets, exp reads the [*, :320] pair in one ACT instr), po + psm
    1 bank each = 8 banks.
"""

import sys

if "/opt/trn_rl_repo" not in sys.path:
    sys.path.insert(0, "/opt/trn_rl_repo")

import ml_dtypes
import numpy as np

# --- problem constants (hardcoded, must match the grading reference) ---
B, N, C = 32, 1280, 256
H, KD, D = 8, 64, 128          # heads, key dim, value dim per head
NQ = 320                       # subsampled sequence length
OUT = 384
NCORES = 8
BPC = B // NCORES              # batches per core
EPS = 1e-5
NCH = N // 128                 # 10 n-chunks of 128
GRP = 2                        # scores psum group size (n-chunks per group)

# per-head engine tuning: bias add on PE (fp8 DoubleRow) vs DVE (exp-bias mult)
BIAS_PE = [True] * 8
# per-head: softmax sums via 10 PE ones-matmuls vs DVE accumulate + 1 matmul
SUMS_PE = [False] * 8

_PE_HEADS = [h for h in range(H) if BIAS_PE[h]]
_DVE_HEADS = [h for h in range(H) if not BIAS_PE[h]]
_PE_SLOT = {h: i for i, h in enumerate(_PE_HEADS)}
_DVE_SLOT = {h: i for i, h in enumerate(_DVE_HEADS)}

_SUB_IDX = np.concatenate([
    (np.arange(32)[::2][:, None] * 32 + np.arange(32)[::2][None, :]).reshape(-1),
    1024 + (np.arange(16)[::2][:, None] * 16 + np.arange(16)[::2][None, :]).reshape(-1),
])  # [320] subsample row gather


def _prep(inputs):
    """Host-side: fold BN into weights, reorder channels, shard over cores."""
    f32 = np.float32
    x = np.asarray(inputs["x"], f32)
    g_kv, b_kv = np.asarray(inputs["g_kv"], f32), np.asarray(inputs["b_kv"], f32)
    rm_kv, rv_kv = np.asarray(inputs["rm_kv"], f32), np.asarray(inputs["rv_kv"], f32)
    g_q, b_q = np.asarray(inputs["g_q"], f32), np.asarray(inputs["b_q"], f32)
    rm_q, rv_q = np.asarray(inputs["rm_q"], f32), np.asarray(inputs["rv_q"], f32)
    g_p, b_p = np.asarray(inputs["g_p"], f32), np.asarray(inputs["b_p"], f32)
    rm_p, rv_p = np.asarray(inputs["rm_p"], f32), np.asarray(inputs["rv_p"], f32)
    W_kv = np.asarray(inputs["W_kv"], f32)
    W_q = np.asarray(inputs["W_q"], f32)
    W_p = np.asarray(inputs["W_p"], f32)
    attn_bias = np.asarray(inputs["attn_bias"], f32)
    bias_idxs = np.asarray(inputs["bias_idxs"])

    s_kv = g_kv / np.sqrt(rv_kv + EPS)
    Wkv_f = W_kv * s_kv[:, None]
    bkv_f = b_kv - rm_kv * s_kv
    kidx = np.concatenate([np.arange(h * 192, h * 192 + KD) for h in range(H)])
    vidx = np.concatenate([np.arange(h * 192 + KD, (h + 1) * 192) for h in range(H)])
    wkt = np.ascontiguousarray(Wkv_f[kidx].T).reshape(2, 128, 512)     # [c,128][512 kch]
    wvt = np.ascontiguousarray(Wkv_f[vidx].T).reshape(2, 128, 1024)
    bvd = np.ascontiguousarray(bkv_f[vidx].reshape(8, 128).T)          # [128, H]

    scale = KD ** -0.5
    s_q = g_q / np.sqrt(rv_q + EPS)
    wqt = np.ascontiguousarray((W_q * (s_q * scale)[:, None]).T).reshape(2, 128, 512)
    bq = np.ascontiguousarray(((b_q - rm_q * s_q) * scale).reshape(4, 128).T)

    s_p = g_p / np.sqrt(rv_p + EPS)
    wpt = np.ascontiguousarray((W_p * s_p[:, None]).T / 6.0).reshape(
        8, 128, OUT).astype(ml_dtypes.bfloat16)
    bps = np.ascontiguousarray(np.broadcast_to(b_p - rm_p * s_p, (128, OUT))).astype(np.float32)

    biasT = attn_bias[:, bias_idxs].transpose(0, 2, 1)                 # [H, N, NQ]
    bias_cpq = biasT.reshape(H, NCH, 128, NQ).transpose(0, 2, 1, 3)    # [H,128,NCH,NQ]
    f8 = ml_dtypes.float8_e4m3
    # bias fp8, stored once per head; the DoubleRow identity pair (I,0)/(0,I)
    # selects one chunk of an adjacent pair per instruction
    bt8 = np.ascontiguousarray(bias_cpq).astype(f8)                    # [H,128,NCH,NQ]

    identp = np.zeros((128, 2, 2, 128), f8)
    identp[np.arange(128), 0, 0, np.arange(128)] = 1.0
    identp[np.arange(128), 1, 1, np.arange(128)] = 1.0

    wv8h = wvt.astype(f8)
    wv8l = (wvt - wv8h.astype(np.float32)).astype(f8)

    xs = x[:, _SUB_IDX, :]                                             # [B, NQ, C]
    in_maps = []
    for i in range(NCORES):
        sl = slice(i * BPC, (i + 1) * BPC)
        xt = np.ascontiguousarray(x[sl].transpose(0, 2, 1)).reshape(BPC, 2, 128, N)
        x8h = xt.astype(f8)
        x8l = (xt - x8h.astype(np.float32)).astype(f8)
        xst = np.ascontiguousarray(xs[sl].transpose(0, 2, 1)).reshape(BPC, 2, 128, NQ)
        in_maps.append({
            "xt": xt, "xst": xst, "x8h": x8h, "x8l": x8l,
            "wv8h": wv8h, "wv8l": wv8l,
            "wkt": wkt, "wvt": wvt, "wqt": wqt, "wpt": wpt,
            "bq": bq, "bv": bvd, "bv3": bvd + 3.0, "bps": bps,
            "bt8": bt8,
            "ones": np.ones((128, 128), ml_dtypes.bfloat16),
            "identp": identp,
        })
    return in_maps


def _body(tc, a, out_ap):
    import concourse.bass as bass  # noqa: F401
    import concourse.mybir as mybir
    from contextlib import ExitStack

    nc = tc.nc
    f32 = mybir.dt.float32
    f32r = mybir.dt.float32r
    bf16 = mybir.dt.bfloat16
    f8e4 = mybir.dt.float8e4
    AF = mybir.ActivationFunctionType
    ALU = mybir.AluOpType
    PM = mybir.MatmulPerfMode

    with ExitStack() as ctx:
        ctx.enter_context(
            nc.allow_low_precision(reason="bf16 o-side + fp8 bias matmuls are deliberate; verified vs fp32 reference")
        )
        singles = ctx.enter_context(tc.tile_pool(name="singles", bufs=1))
        # DMA order matters at startup: first-needed weights first (wk -> q/kT
        # projections of batch 0), small attention-phase tiles later.
        wk = singles.tile([128, 2, 512], f32r)
        nc.sync.dma_start(wk[:, :, 0:128], a["wkt"][:, :, 0:128].rearrange("c p j -> p c j"))
        wq = singles.tile([128, 2, 512], f32r)
        bqs = singles.tile([128, 4], f32)
        wv8h = singles.tile([128, 2, 1024], f8e4)
        wv8l = singles.tile([128, 2, 1024], f8e4)
        wp = singles.tile([128, 8, OUT], bf16)
        bvs = singles.tile([128, H], f32)
        bvs3 = singles.tile([128, H], f32)
        ones = singles.tile([128, 128], bf16)
        identp = singles.tile([128, 2, 2, 128], f8e4)
        bps = singles.tile([128, OUT], f32)
        bt8s = [singles.tile([128, NCH, NQ], f8e4, name=f"bt8h{h}")
                for h in range(H)]

        xt_p = ctx.enter_context(tc.tile_pool(name="xt", bufs=2))
        xst_p = ctx.enter_context(tc.tile_pool(name="xst", bufs=2))
        x8_p = ctx.enter_context(tc.tile_pool(name="x8", bufs=2))
        kt_p = ctx.enter_context(tc.tile_pool(name="kt", bufs=2))
        v_p = ctx.enter_context(tc.tile_pool(name="v", bufs=2))
        qt_p = ctx.enter_context(tc.tile_pool(name="qt", bufs=2))
        e_p = ctx.enter_context(tc.tile_pool(name="e", bufs=6))
        esum_p = ctx.enter_context(tc.tile_pool(name="esum", bufs=1))
        orw_p = ctx.enter_context(tc.tile_pool(name="orw", bufs=2))
        rc_p = ctx.enter_context(tc.tile_pool(name="rc", bufs=2))
        oh_p = ctx.enter_context(tc.tile_pool(name="oh", bufs=2))
        t1_p = ctx.enter_context(tc.tile_pool(name="t1", bufs=2))
        hs_p = ctx.enter_context(tc.tile_pool(name="hs", bufs=2))
        ob_p = ctx.enter_context(tc.tile_pool(name="ob", bufs=2))
        ps_sg = ctx.enter_context(tc.tile_pool(name="ps_sg", bufs=3, space="PSUM"))
        ps_o = ctx.enter_context(tc.tile_pool(name="ps_o", bufs=1, space="PSUM"))
        ps_sum = ctx.enter_context(tc.tile_pool(name="ps_sum", bufs=1, space="PSUM"))

        _wt_n = [0]

        def sg_tile():
            _wt_n[0] += 1
            return ps_sg.tile([128, GRP, 512], f32, tag="sg", name=f"sg{_wt_n[0]}")

        out_flat = out_ap.rearrange("b q o -> (b q) o")

        def dma_x(b, first=False, stagger=False):
            """Issue input DMAs for batch b; returns (xt, xst) tiles."""
            xt = xt_p.tile([128, 2, N], f32r, tag="xt", name=f"xt{b}")
            for ns in range(3):
                if stagger:        # keep the resident-bias stream fed first
                    nc.sync.dma_start(bt8s[2 + ns], a["bt8"][2 + ns])
                n0 = ns * 512
                nsz = min(512, N - n0)
                nc.sync.dma_start(
                    xt[:, :, n0:n0 + nsz],
                    a["xt"][b, :, :, n0:n0 + nsz].rearrange("c p n -> p c n"),
                )
                if first and ns == 0:
                    nc.sync.dma_start(
                        wk[:, :, 128:512],
                        a["wkt"][:, :, 128:512].rearrange("c p j -> p c j"))
                if first and ns == 1:
                    nc.sync.dma_start(wq, a["wqt"].rearrange("c p j -> p c j"))
                    nc.sync.dma_start(bqs, a["bq"])
            if stagger:
                nc.sync.dma_start(bt8s[5], a["bt8"][5])
            xst = xst_p.tile([128, 2, NQ], f32r, tag="xst", name=f"xst{b}")
            nc.sync.dma_start(xst, a["xst"][b].rearrange("c p n -> p c n"))
            x8h = x8_p.tile([128, 2, N], f8e4, tag="x8h", name=f"x8h{b}")
            nc.sync.dma_start(x8h, a["x8h"][b].rearrange("c p n -> p c n"))
            if stagger:
                nc.sync.dma_start(bt8s[6], a["bt8"][6])
            x8l = x8_p.tile([128, 2, N], f8e4, tag="x8l", name=f"x8l{b}")
            nc.sync.dma_start(x8l, a["x8l"][b].rearrange("c p n -> p c n"))
            if stagger:
                nc.sync.dma_start(bt8s[7], a["bt8"][7])
                nc.sync.dma_start(wp, a["wpt"].rearrange("c p j -> p c j"))
                nc.sync.dma_start(bps, a["bps"])
            return xt, xst, x8h, x8l

        def proj_gen(b, xt, xst, x8h, x8l):
            """Yield after each proj psum tile; returns (kt, vt, qt) eagerly."""
            kt = kt_p.tile([128, 4, N], f32r, tag="kt", name=f"kt{b}")
            vt = v_p.tile([128, NCH, 1024], bf16, tag="vt", name=f"vt{b}")
            qt = qt_p.tile([128, 4, NQ], f32r, tag="qt", name=f"qt{b}")

            def emit():
                # kT projection: no bias (softmax-invariant), wide copies
                for pr in range(4):
                    ps = sg_tile()
                    for half in range(2):       # n slices 0:512, 512:1024
                        n0 = half * 512
                        for cc in range(2):
                            nc.tensor.matmul(
                                ps[:, half, :],
                                lhsT=wk[:, cc, pr * 128:(pr + 1) * 128],
                                rhs=xt[:, cc, n0:n0 + 512],
                                start=(cc == 0), stop=(cc == 1),
                            )
                    if pr < 2:
                        nc.vector.tensor_copy(
                            kt[:, pr, 0:1024], ps.rearrange("p g j -> p (g j)"),
                        )
                    else:
                        nc.scalar.copy(
                            kt[:, pr, 0:1024], ps.rearrange("p g j -> p (g j)"),
                        )
                    yield
                ps = sg_tile()                  # 256-col tails, two prs per tile
                for prh in range(2):
                    for j in range(2):
                        pr = 2 * prh + j
                        for cc in range(2):
                            nc.tensor.matmul(
                                ps[:, j, :256] if prh == 0 else ps[:, j, 256:512],
                                lhsT=wk[:, cc, pr * 128:(pr + 1) * 128],
                                rhs=xt[:, cc, 1024:N],
                                start=(cc == 0), stop=(cc == 1),
                            )
                        if prh == 0:
                            nc.scalar.copy(kt[:, pr, 1024:N], ps[:, j, :256])
                        else:
                            nc.scalar.copy(kt[:, pr, 1024:N], ps[:, j, 256:512])
                yield
                # q projection: 2 prs per tile, fused bias on DVE
                for half in range(2):
                    ps = sg_tile()
                    for j in range(2):
                        pr = 2 * half + j
                        for cc in range(2):
                            nc.tensor.matmul(
                                ps[:, j, :NQ],
                                lhsT=wq[:, cc, pr * 128:(pr + 1) * 128],
                                rhs=xst[:, cc, :],
                                start=(cc == 0), stop=(cc == 1),
                            )
                    nc.vector.tensor_tensor(
                        qt[:, 2 * half:2 * half + 2, :], ps[:, :, :NQ],
                        bqs[:, 2 * half:2 * half + 2].to_broadcast((128, 2, NQ)),
                        ALU.add,
                    )
                    yield
                # v projection: fp8 hi/lo DoubleRow (K=256 per pass, 3 passes)
                for cn in range(NCH):
                    ps = sg_tile()
                    for hf in range(2):
                        for pi, (xx, ww) in enumerate(
                                ((x8h, wv8h), (x8l, wv8h), (x8h, wv8l))):
                            nc.tensor.matmul(
                                ps[:, hf, :],
                                lhsT=xx[:, :, cn * 128:(cn + 1) * 128],
                                rhs=ww[:, :, hf * 512:(hf + 1) * 512],
                                start=(pi == 0), stop=(pi == 2),
                                perf_mode=PM.DoubleRow,
                            )
                    if cn % 2 == 0:
                        nc.vector.tensor_copy(
                            vt[:, cn, :], ps.rearrange("p g j -> p (g j)"))
                    else:
                        nc.scalar.copy(
                            vt[:, cn, :], ps.rearrange("p g j -> p (g j)"))
                    yield

            return kt, vt, qt, emit()

        def attention(b, h, kt, vt, qt, t2):
            # generator: yields after each score group so the driver can
            # weave projection tiles of the next batch between groups
            yield
            pr, p0 = h // 2, 64 * (h % 2)
            on_pe = BIAS_PE[h]
            bt8 = bt8s[h]
            po = ps_o.tile([128, NQ], f32, tag="po", name=f"po_{b}_{h}")
            e_tiles = []
            for g in range(NCH // GRP):
                sg = sg_tile()
                for j in range(GRP):
                    c = GRP * g + j
                    nc.tensor.matmul(
                        sg[:, j, :NQ],
                        lhsT=kt[p0:p0 + 64, pr, c * 128:(c + 1) * 128],
                        rhs=qt[p0:p0 + 64, pr, :],
                        start=True, stop=(not on_pe),
                    )
                    if on_pe:
                        nc.tensor.matmul(
                            sg[:, j, :NQ],
                            lhsT=identp[:, j, :, :],
                            rhs=bt8[:, GRP * g:GRP * (g + 1), :],
                            start=False, stop=True,
                            perf_mode=PM.DoubleRow,
                        )
                e = e_p.tile([128, GRP, NQ], bf16)
                nc.scalar.activation(e, sg[:, :, :NQ], AF.Exp)
                e_tiles.append(e)
                for j in range(GRP):
                    c = GRP * g + j
                    nc.tensor.matmul(
                        po[:, :NQ],
                        lhsT=vt[:, c, h * 128:(h + 1) * 128],
                        rhs=e[:, j, :],
                        start=(c == 0), stop=(c == NCH - 1),
                    )
                yield
            # softmax denominators
            psm = ps_sum.tile([128, NQ], f32, tag="psm", name=f"psm_{b}_{h}")
            if SUMS_PE[h]:
                for g in range(NCH // GRP):
                    for j in range(GRP):
                        c = GRP * g + j
                        nc.tensor.matmul(
                            psm,
                            lhsT=ones,
                            rhs=e_tiles[g][:, j, :],
                            start=(c == 0), stop=(c == NCH - 1),
                        )
            else:
                acc = e_tiles[1]
                nc.vector.tensor_tensor(acc, e_tiles[0], e_tiles[1], ALU.add)
                for g in range(2, NCH // GRP):
                    nc.vector.tensor_tensor(acc, acc, e_tiles[g], ALU.add)
                esum = esum_p.tile([128, NQ], bf16)
                nc.vector.tensor_tensor(esum, acc[:, 0, :], acc[:, 1, :], ALU.add)
                nc.tensor.matmul(psm, lhsT=ones, rhs=esum, start=True, stop=True)
            rc = rc_p.tile([128, NQ], bf16)
            nc.vector.reciprocal(rc, psm)
            oraw = orw_p.tile([128, NQ], bf16, tag="oraw", name=f"oraw_{b}_{h}")
            nc.vector.tensor_copy(oraw, po)
            oh = oh_p.tile([128, NQ], bf16)
            nc.gpsimd.tensor_tensor(oh, oraw, rc, ALU.mult)
            # hswish: t = min(Relu(o + 3 + bv), 6);  th = (o + bv) * t
            t1 = t1_p.tile([128, NQ], bf16)
            nc.gpsimd.tensor_scalar(t1, oh, bvs3[:, h:h + 1], 0.0, ALU.add, ALU.max)
            nc.gpsimd.tensor_scalar(t1, t1, 6.0, None, ALU.min)
            obv = t1_p.tile([128, NQ], bf16, tag="obv", name=f"obv_{b}_{h}")
            nc.gpsimd.tensor_scalar(obv, oh, bvs[:, h:h + 1], None, ALU.add)
            nc.gpsimd.tensor_tensor(t2[:, h, b % 2, :], obv, t1, ALU.mult)

        def out_proj(b, t2, qcs=range(5)):
            for qc in qcs:
                r0 = (b - 1) * NQ + qc * 128
                ps = sg_tile()
                for dc in range(8):
                    nc.tensor.matmul(
                        ps[:, 0, :OUT],
                        lhsT=t2[:, dc, :, :].rearrange(
                            "p bb q -> p (bb q)")[:, qc * 128:(qc + 1) * 128],
                        rhs=wp[:, dc, :],
                        start=(dc == 0), stop=(dc == 7),
                    )
                ob = ob_p.tile([128, OUT], f32)
                nc.vector.tensor_tensor(ob, ps[:, 0, :OUT], bps, ALU.add)
                nc.sync.dma_start(out_flat[r0:r0 + 128, :], ob)
                yield

        # prologue: batch 0 inputs + weights; proj(0) up to attention-ready
        xt0, xst0, x8h0, x8l0 = dma_x(0, first=True)
        nc.sync.dma_start(identp, a["identp"])
        nc.sync.dma_start(wv8h, a["wv8h"].rearrange("c p j -> p c j"))
        nc.sync.dma_start(bt8s[0], a["bt8"][0])
        nc.sync.dma_start(wv8l, a["wv8l"].rearrange("c p j -> p c j"))
        nc.sync.dma_start(bt8s[1], a["bt8"][1])
        nc.sync.dma_start(ones, a["ones"])
        nc.sync.dma_start(bvs, a["bv"])
        nc.sync.dma_start(bvs3, a["bv3"])

        # warm up the PE p-state during the input-DMA wait: dummy matmuls
        # on a memset scratch tile so the ramp starts at ~0.5us, reaching
        # full clock before the first real projection matmul
        warm = sg_tile()
        for i in range(8):
            nc.tensor.matmul(
                warm[:, 0, :256],
                lhsT=wk[:, 0, 0:128],
                rhs=wk[:, :, 0:128],
                start=True, stop=True,
            )

        kt, vt, qt, gen0 = proj_gen(0, xt0, xst0, x8h0, x8l0)
        for _ in gen0:          # batch 0 proj must fully precede its attention
            pass
        pending = []
        t2 = None
        nxt = None
        for b in range(BPC):
            if b % 2 == 0:
                t2 = hs_p.tile([128, H, 2, NQ], bf16, tag="t2", name=f"t2_{b}")
            # interleave remaining proj tiles (this batch's tail + next batch)
            if b + 1 < BPC:
                xtn, xstn, x8hn, x8ln = dma_x(b + 1, stagger=(b == 0))
                nxt = proj_gen(b + 1, xtn, xstn, x8hn, x8ln)
                pending.append(nxt[3])
            for h in range(H):
                for gi, _ in enumerate(attention(b, h, kt, vt, qt, t2)):
                    if gi in (1, 2, 3, 4, 5):
                        while pending:
                            if next(pending[0], "done") == "done":
                                pending.pop(0)
                            else:
                                break
            while pending:
                if next(pending[0], "done") == "done":
                    pending.pop(0)
                else:
                    break
            if pending:
                for _ in pending[0]:
                    pass
                pending.pop(0)
            if b % 2 == 1:
                if b + 1 < BPC:
                    pending.append(out_proj(b, t2))  # interleave with next batch
                else:
                    for _ in out_proj(b, t2):
                        pass
            if nxt is not None:
                kt, vt, qt = nxt[0], nxt[1], nxt[2]
                nxt = None


def build():
    import concourse.mybir as mybir
    import concourse.tile as tile
    from concourse import bacc

    nc = bacc.Bacc("TRN2", target_bir_lowering=False, debug=False)
    f32, bf16 = mybir.dt.float32, mybir.dt.bfloat16
    f8e4 = mybir.dt.float8e4
    a = {}

    def din(name, shape, dt=f32):
        a[name] = nc.dram_tensor(name, shape, dt, kind="ExternalInput").ap()

    f32r = mybir.dt.float32r
    din("xt", [BPC, 2, 128, N], f32r)
    din("xst", [BPC, 2, 128, NQ], f32r)
    din("wkt", [2, 128, 512], f32r)
    din("wvt", [2, 128, 1024], f32r)
    din("x8h", [BPC, 2, 128, N], f8e4)
    din("x8l", [BPC, 2, 128, N], f8e4)
    din("wv8h", [2, 128, 1024], f8e4)
    din("wv8l", [2, 128, 1024], f8e4)
    din("wqt", [2, 128, 512], f32r)
    din("wpt", [8, 128, OUT], bf16)
    din("bq", [128, 4])
    din("bv", [128, H])
    din("bv3", [128, H])
    din("bps", [128, OUT])
    din("bt8", [H, 128, NCH, NQ], f8e4)
    din("ones", [128, 128], bf16)
    din("identp", [128, 2, 2, 128], f8e4)
    out_ap = nc.dram_tensor("out", [BPC, NQ, OUT], f32, kind="ExternalOutput").ap()

    with tile.TileContext(nc) as tc:
        _body(tc, a, out_ap)
    nc.compile()
    return nc


_NC_CACHE = None


def _get_nc():
    global _NC_CACHE
    if _NC_CACHE is None:
        _NC_CACHE = build()
    return _NC_CACHE


def kernel(**inputs):
    from concourse.bass_utils import run_bass_kernel_spmd

    in_maps = _prep(inputs)
    nc = _get_nc()
    res = run_bass_kernel_spmd(nc, in_maps, list(range(NCORES)))
    out = np.concatenate([res.results[i]["out"] for i in range(NCORES)], axis=0)
    return np.ascontiguousarray(out, dtype=np.float32)


if __name__ == "__main__":
    rng = np.random.default_rng(0)
    print("smoke: building bass module...")
    nc = build()
    print("built ok:", sum(len(bb.instructions) for bb in nc.m.functions[0].blocks), "instructions")


# revision 10
# speedup vs baseline: 1.3244x; 1.0043x over previous
"""AttentionSubsample Trainium2 kernel.

Full (unsharded) inputs in, full output out. Data-parallel over batch:
32 batches -> 8 NeuronCores x 4 batches each. Weights/biases replicated.

Engine-balance design (cost-model 208.1us/core, vs 275.6us v1 baseline):
  - k-channel BN bias dropped entirely: softmax over n is invariant to
    per-q shifts and (k+bk)@q shifts every key n equally.
  - score bias added pre-exp on the PE as fp8(e4m3) DoubleRow identity
    matmuls (0.5 cyc/row): lhsT=(I,0)/(0,I) selects one chunk of an
    adjacent bias-chunk pair, so the bias stays resident in SBUF stored
    once (3.2KB/partition/head, loaded one time, no per-batch DMA).
  - v projection as fp8 hi/lo split (x = x8h + x8l, Wv = w8h + w8l) with
    three K=256 DoubleRow passes per psum tile, dropping the lo*lo term:
    ~2.7x fewer PE cycles than f32r at bf16-level accuracy. kT/q stay
    f32r: their quantization noise would amplify through exp by sqrt(d).
  - softmax sums: e-tiles accumulated on DVE (bf16 2x mode, in-place
    chain) + one ones-matmul per head instead of 10 PE ones-matmuls.
  - hswish on Pool/DVE: t = min(Relu(o+3+bv), 6) via Pool tensor_scalar
    ops, th = (o+bv)*t via Pool scalar_tensor_tensor; normalize mult on
    Pool; bv folds out of attn@v (softmax rows sum to 1).
  - psum->sbuf copies split across ACT/DVE (GPSIMD cannot touch PSUM on
    real hw); out-proj bias fused into the DVE psum->sbuf add.
  - software pipelining: batch b+1's kT/q/v projection psum tiles are
    emitted interleaved between batch b's attention score groups (one
    tile after every group, via generators), and the pair output
    projection interleaves with the following batch, keeping the PE fed
    through the shared psum-pool rotation.
  - PSUM: scores pool 3x[128,2,512] (chunk pairs at bank-aligned 512
    offsets, exp reads the [*, :320] pair in one ACT instr), po + psm
    1 bank each = 8 banks.
"""

import sys

if "/opt/trn_rl_repo" not in sys.path:
    sys.path.insert(0, "/opt/trn_rl_repo")

import ml_dtypes
import numpy as np

# --- problem constants (hardcoded, must match the grading reference) ---
B, N, C = 32, 1280, 256
H, KD, D = 8, 64, 128          # heads, key dim, value dim per head
NQ = 320                       # subsampled sequence length
OUT = 384
NCORES = 8
BPC = B // NCORES              # batches per core
EPS = 1e-5
NCH = N // 128                 # 10 n-chunks of 128
GRP = 2                        # scores psum group size (n-chunks per group)

# per-head engine tuning: bias add on PE (fp8 DoubleRow) vs DVE (exp-bias mult)
BIAS_PE = [True] * 8
# per-head: softmax sums via 10 PE ones-matmuls vs DVE accumulate + 1 matmul
SUMS_PE = [False] * 8

_PE_HEADS = [h for h in range(H) if BIAS_PE[h]]
_DVE_HEADS = [h for h in range(H) if not BIAS_PE[h]]
_PE_SLOT = {h: i for i, h in enumerate(_PE_HEADS)}
_DVE_SLOT = {h: i for i, h in enumerate(_DVE_HEADS)}

_SUB_IDX = np.concatenate([
    (np.arange(32)[::2][:, None] * 32 + np.arange(32)[::2][None, :]).reshape(-1),
    1024 + (np.arange(16)[::2][:, None] * 16 + np.arange(16)[::2][None, :]).reshape(-1),
])  # [320] subsample row gather


def _prep(inputs):
    """Host-side: fold BN into weights, reorder channels, shard over cores."""
    f32 = np.float32
    x = np.asarray(inputs["x"], f32)
    g_kv, b_kv = np.asarray(inputs["g_kv"], f32), np.asarray(inputs["b_kv"], f32)
    rm_kv, rv_kv = np.asarray(inputs["rm_kv"], f32), np.asarray(inputs["rv_kv"], f32)
    g_q, b_q = np.asarray(inputs["g_q"], f32), np.asarray(inputs["b_q"], f32)
    rm_q, rv_q = np.asarray(inputs["rm_q"], f32), np.asarray(inputs["rv_q"], f32)
    g_p, b_p = np.asarray(inputs["g_p"], f32), np.asarray(inputs["b_p"], f32)
    rm_p, rv_p = np.asarray(inputs["rm_p"], f32), np.asarray(inputs["rv_p"], f32)
    W_kv = np.asarray(inputs["W_kv"], f32)
    W_q = np.asarray(inputs["W_q"], f32)
    W_p = np.asarray(inputs["W_p"], f32)
    attn_bias = np.asarray(inputs["attn_bias"], f32)
    bias_idxs = np.asarray(inputs["bias_idxs"])

    s_kv = g_kv / np.sqrt(rv_kv + EPS)
    Wkv_f = W_kv * s_kv[:, None]
    bkv_f = b_kv - rm_kv * s_kv
    kidx = np.concatenate([np.arange(h * 192, h * 192 + KD) for h in range(H)])
    vidx = np.concatenate([np.arange(h * 192 + KD, (h + 1) * 192) for h in range(H)])
    wkt = np.ascontiguousarray(Wkv_f[kidx].T).reshape(2, 128, 512)     # [c,128][512 kch]
    wvt = np.ascontiguousarray(Wkv_f[vidx].T).reshape(2, 128, 1024)
    bvd = np.ascontiguousarray(bkv_f[vidx].reshape(8, 128).T)          # [128, H]

    scale = KD ** -0.5
    s_q = g_q / np.sqrt(rv_q + EPS)
    wqt = np.ascontiguousarray((W_q * (s_q * scale)[:, None]).T).reshape(2, 128, 512)
    bq = np.ascontiguousarray(((b_q - rm_q * s_q) * scale).reshape(4, 128).T)

    s_p = g_p / np.sqrt(rv_p + EPS)
    wpt = np.ascontiguousarray((W_p * s_p[:, None]).T / 6.0).reshape(
        8, 128, OUT).astype(ml_dtypes.bfloat16)
    bps = np.ascontiguousarray(np.broadcast_to(b_p - rm_p * s_p, (128, OUT))).astype(np.float32)

    biasT = attn_bias[:, bias_idxs].transpose(0, 2, 1)                 # [H, N, NQ]
    bias_cpq = biasT.reshape(H, NCH, 128, NQ).transpose(0, 2, 1, 3)    # [H,128,NCH,NQ]
    f8 = ml_dtypes.float8_e4m3
    # bias fp8, stored once per head; the DoubleRow identity pair (I,0)/(0,I)
    # selects one chunk of an adjacent pair per instruction
    bt8 = np.ascontiguousarray(bias_cpq).astype(f8)                    # [H,128,NCH,NQ]

    identp = np.zeros((128, 2, 2, 128), f8)
    identp[np.arange(128), 0, 0, np.arange(128)] = 1.0
    identp[np.arange(128), 1, 1, np.arange(128)] = 1.0

    wv8h = wvt.astype(f8)
    wv8l = (wvt - wv8h.astype(np.float32)).astype(f8)

    xs = x[:, _SUB_IDX, :]                                             # [B, NQ, C]
    in_maps = []
    for i in range(NCORES):
        sl = slice(i * BPC, (i + 1) * BPC)
        xt = np.ascontiguousarray(x[sl].transpose(0, 2, 1)).reshape(BPC, 2, 128, N)
        x8h = xt.astype(f8)
        x8l = (xt - x8h.astype(np.float32)).astype(f8)
        xst = np.ascontiguousarray(xs[sl].transpose(0, 2, 1)).reshape(BPC, 2, 128, NQ)
        in_maps.append({
            "xt": xt, "xst": xst, "x8h": x8h, "x8l": x8l,
            "wv8h": wv8h, "wv8l": wv8l,
            "wkt": wkt, "wvt": wvt, "wqt": wqt, "wpt": wpt,
            "bq": bq, "bv": bvd, "bv3": bvd + 3.0, "bps": bps,
            "bt8": bt8,
            "ones": np.ones((128, 128), ml_dtypes.bfloat16),
            "identp": identp,
        })
    return in_maps


def _body(tc, a, out_ap):
    import concourse.bass as bass  # noqa: F401
    import concourse.mybir as mybir
    from contextlib import ExitStack

    nc = tc.nc
    f32 = mybir.dt.float32
    f32r = mybir.dt.float32r
    bf16 = mybir.dt.bfloat16
    f8e4 = mybir.dt.float8e4
    AF = mybir.ActivationFunctionType
    ALU = mybir.AluOpType
    PM = mybir.MatmulPerfMode

    with ExitStack() as ctx:
        ctx.enter_context(
            nc.allow_low_precision(reason="bf16 o-side + fp8 bias matmuls are deliberate; verified vs fp32 reference")
        )
        singles = ctx.enter_context(tc.tile_pool(name="singles", bufs=1))
        # DMA order matters at startup: first-needed weights first (wk -> q/kT
        # projections of batch 0), small attention-phase tiles later.
        wk = singles.tile([128, 2, 512], f32r)
        nc.sync.dma_start(wk[:, :, 0:128], a["wkt"][:, :, 0:128].rearrange("c p j -> p c j"))
        wq = singles.tile([128, 2, 512], f32r)
        bqs = singles.tile([128, 4], f32)
        wv8h = singles.tile([128, 2, 1024], f8e4)
        wv8l = singles.tile([128, 2, 1024], f8e4)
        wp = singles.tile([128, 8, OUT], bf16)
        bvs = singles.tile([128, H], f32)
        bvs3 = singles.tile([128, H], f32)
        ones = singles.tile([128, 128], bf16)
        identp = singles.tile([128, 2, 2, 128], f8e4)
        bps = singles.tile([128, OUT], f32)
        bt8s = [singles.tile([128, NCH, NQ], f8e4, name=f"bt8h{h}")
                for h in range(H)]

        xt_p = ctx.enter_context(tc.tile_pool(name="xt", bufs=1))
        xst_p = ctx.enter_context(tc.tile_pool(name="xst", bufs=1))
        x8_p = ctx.enter_context(tc.tile_pool(name="x8", bufs=1))
        kt_p = ctx.enter_context(tc.tile_pool(name="kt", bufs=2))
        v_p = ctx.enter_context(tc.tile_pool(name="v", bufs=2))
        qt_p = ctx.enter_context(tc.tile_pool(name="qt", bufs=3))
        e_p = ctx.enter_context(tc.tile_pool(name="e", bufs=8))
        esum_p = ctx.enter_context(tc.tile_pool(name="esum", bufs=2))
        orw_p = ctx.enter_context(tc.tile_pool(name="orw", bufs=3))
        rc_p = ctx.enter_context(tc.tile_pool(name="rc", bufs=3))
        oh_p = ctx.enter_context(tc.tile_pool(name="oh", bufs=3))
        t1_p = ctx.enter_context(tc.tile_pool(name="t1", bufs=3))
        hs_p = ctx.enter_context(tc.tile_pool(name="hs", bufs=2))
        ob_p = ctx.enter_context(tc.tile_pool(name="ob", bufs=3))
        ps_sg = ctx.enter_context(tc.tile_pool(name="ps_sg", bufs=3, space="PSUM"))
        ps_o = ctx.enter_context(tc.tile_pool(name="ps_o", bufs=1, space="PSUM"))
        ps_sum = ctx.enter_context(tc.tile_pool(name="ps_sum", bufs=1, space="PSUM"))

        _wt_n = [0]

        def sg_tile():
            _wt_n[0] += 1
            return ps_sg.tile([128, GRP, 512], f32, tag="sg", name=f"sg{_wt_n[0]}")

        out_flat = out_ap.rearrange("b q o -> (b q) o")

        def dma_x(b, first=False, stagger=False):
            """Issue input DMAs for batch b; returns (xt, xst) tiles."""
            xt = xt_p.tile([128, 2, N], f32r, tag="xt", name=f"xt{b}")
            for ns in range(3):
                if stagger:        # keep the resident-bias stream fed first
                    nc.sync.dma_start(bt8s[2 + ns], a["bt8"][2 + ns])
                n0 = ns * 512
                nsz = min(512, N - n0)
                nc.sync.dma_start(
                    xt[:, :, n0:n0 + nsz],
                    a["xt"][b, :, :, n0:n0 + nsz].rearrange("c p n -> p c n"),
                )
                if first and ns == 0:
                    nc.sync.dma_start(
                        wk[:, :, 128:512],
                        a["wkt"][:, :, 128:512].rearrange("c p j -> p c j"))
                if first and ns == 1:
                    nc.sync.dma_start(wq, a["wqt"].rearrange("c p j -> p c j"))
                    nc.sync.dma_start(bqs, a["bq"])
            if stagger:
                nc.sync.dma_start(bt8s[5], a["bt8"][5])
            xst = xst_p.tile([128, 2, NQ], f32r, tag="xst", name=f"xst{b}")
            nc.sync.dma_start(xst, a["xst"][b].rearrange("c p n -> p c n"))
            x8h = x8_p.tile([128, 2, N], f8e4, tag="x8h", name=f"x8h{b}")
            nc.sync.dma_start(x8h, a["x8h"][b].rearrange("c p n -> p c n"))
            if stagger:
                nc.sync.dma_start(bt8s[6], a["bt8"][6])
            x8l = x8_p.tile([128, 2, N], f8e4, tag="x8l", name=f"x8l{b}")
            nc.sync.dma_start(x8l, a["x8l"][b].rearrange("c p n -> p c n"))
            if stagger:
                nc.sync.dma_start(bt8s[7], a["bt8"][7])
                nc.sync.dma_start(wp, a["wpt"].rearrange("c p j -> p c j"))
                nc.sync.dma_start(bps, a["bps"])
            return xt, xst, x8h, x8l

        def proj_gen(b, xt, xst, x8h, x8l):
            """Yield after each proj psum tile; returns (kt, vt, qt) eagerly."""
            kt = kt_p.tile([128, 4, N], f32r, tag="kt", name=f"kt{b}")
            vt = v_p.tile([128, NCH, 1024], bf16, tag="vt", name=f"vt{b}")
            qt = qt_p.tile([128, 4, NQ], f32r, tag="qt", name=f"qt{b}")

            def emit():
                # kT projection: no bias (softmax-invariant), wide copies
                for pr in range(4):
                    ps = sg_tile()
                    for half in range(2):       # n slices 0:512, 512:1024
                        n0 = half * 512
                        for cc in range(2):
                            nc.tensor.matmul(
                                ps[:, half, :],
                                lhsT=wk[:, cc, pr * 128:(pr + 1) * 128],
                                rhs=xt[:, cc, n0:n0 + 512],
                                start=(cc == 0), stop=(cc == 1),
                            )
                    if pr < 2:
                        nc.vector.tensor_copy(
                            kt[:, pr, 0:1024], ps.rearrange("p g j -> p (g j)"),
                        )
                    else:
                        nc.scalar.copy(
                            kt[:, pr, 0:1024], ps.rearrange("p g j -> p (g j)"),
                        )
                    yield
                ps = sg_tile()                  # 256-col tails, two prs per tile
                for prh in range(2):
                    for j in range(2):
                        pr = 2 * prh + j
                        for cc in range(2):
                            nc.tensor.matmul(
                                ps[:, j, :256] if prh == 0 else ps[:, j, 256:512],
                                lhsT=wk[:, cc, pr * 128:(pr + 1) * 128],
                                rhs=xt[:, cc, 1024:N],
                                start=(cc == 0), stop=(cc == 1),
                            )
                        if prh == 0:
                            nc.scalar.copy(kt[:, pr, 1024:N], ps[:, j, :256])
                        else:
                            nc.scalar.copy(kt[:, pr, 1024:N], ps[:, j, 256:512])
                yield
                # q projection: 2 prs per tile, fused bias on DVE
                for half in range(2):
                    ps = sg_tile()
                    for j in range(2):
                        pr = 2 * half + j
                        for cc in range(2):
                            nc.tensor.matmul(
                                ps[:, j, :NQ],
                                lhsT=wq[:, cc, pr * 128:(pr + 1) * 128],
                                rhs=xst[:, cc, :],
                                start=(cc == 0), stop=(cc == 1),
                            )
                    nc.vector.tensor_tensor(
                        qt[:, 2 * half:2 * half + 2, :], ps[:, :, :NQ],
                        bqs[:, 2 * half:2 * half + 2].to_broadcast((128, 2, NQ)),
                        ALU.add,
                    )
                    yield
                # v projection: fp8 hi/lo DoubleRow (K=256 per pass, 3 passes)
                for cn in range(NCH):
                    ps = sg_tile()
                    for hf in range(2):
                        for pi, (xx, ww) in enumerate(
                                ((x8h, wv8h), (x8l, wv8h), (x8h, wv8l))):
                            nc.tensor.matmul(
                                ps[:, hf, :],
                                lhsT=xx[:, :, cn * 128:(cn + 1) * 128],
                                rhs=ww[:, :, hf * 512:(hf + 1) * 512],
                                start=(pi == 0), stop=(pi == 2),
                                perf_mode=PM.DoubleRow,
                            )
                    if cn % 2 == 0:
                        nc.vector.tensor_copy(
                            vt[:, cn, :], ps.rearrange("p g j -> p (g j)"))
                    else:
                        nc.scalar.copy(
                            vt[:, cn, :], ps.rearrange("p g j -> p (g j)"))
                    yield

            return kt, vt, qt, emit()

        def attention(b, h, kt, vt, qt, t2):
            # generator: yields after each score group so the driver can
            # weave projection tiles of the next batch between groups
            yield
            pr, p0 = h // 2, 64 * (h % 2)
            on_pe = BIAS_PE[h]
            bt8 = bt8s[h]
            po = ps_o.tile([128, NQ], f32, tag="po", name=f"po_{b}_{h}")
            e_tiles = []
            for g in range(NCH // GRP):
                sg = sg_tile()
                for j in range(GRP):
                    c = GRP * g + j
                    nc.tensor.matmul(
                        sg[:, j, :NQ],
                        lhsT=kt[p0:p0 + 64, pr, c * 128:(c + 1) * 128],
                        rhs=qt[p0:p0 + 64, pr, :],
                        start=True, stop=(not on_pe),
                    )
                    if on_pe:
                        nc.tensor.matmul(
                            sg[:, j, :NQ],
                            lhsT=identp[:, j, :, :],
                            rhs=bt8[:, GRP * g:GRP * (g + 1), :],
                            start=False, stop=True,
                            perf_mode=PM.DoubleRow,
                        )
                e = e_p.tile([128, GRP, NQ], bf16)
                nc.scalar.activation(e, sg[:, :, :NQ], AF.Exp)
                e_tiles.append(e)
                for j in range(GRP):
                    c = GRP * g + j
                    nc.tensor.matmul(
                        po[:, :NQ],
                        lhsT=vt[:, c, h * 128:(h + 1) * 128],
                        rhs=e[:, j, :],
                        start=(c == 0), stop=(c == NCH - 1),
                    )
                yield
            # softmax denominators
            psm = ps_sum.tile([128, NQ], f32, tag="psm", name=f"psm_{b}_{h}")
            if SUMS_PE[h]:
                for g in range(NCH // GRP):
                    for j in range(GRP):
                        c = GRP * g + j
                        nc.tensor.matmul(
                            psm,
                            lhsT=ones,
                            rhs=e_tiles[g][:, j, :],
                            start=(c == 0), stop=(c == NCH - 1),
                        )
            else:
                t12 = e_tiles[1]
                nc.vector.tensor_tensor(t12, e_tiles[0], e_tiles[1], ALU.add)
                t34 = e_tiles[3]
                nc.vector.tensor_tensor(t34, e_tiles[2], e_tiles[3], ALU.add)
                nc.vector.tensor_tensor(t12, t12, t34, ALU.add)
                nc.vector.tensor_tensor(t12, t12, e_tiles[4], ALU.add)
                esum = esum_p.tile([128, NQ], bf16)
                nc.vector.tensor_tensor(esum, t12[:, 0, :], t12[:, 1, :], ALU.add)
                nc.tensor.matmul(psm, lhsT=ones, rhs=esum, start=True, stop=True)
            rc = rc_p.tile([128, NQ], bf16)
            nc.vector.reciprocal(rc, psm)
            oraw = orw_p.tile([128, NQ], bf16, tag="oraw", name=f"oraw_{b}_{h}")
            nc.vector.tensor_copy(oraw, po)
            oh = oh_p.tile([128, NQ], bf16)
            nc.gpsimd.tensor_tensor(oh, oraw, rc, ALU.mult)
            # hswish: t = min(Relu(o + 3 + bv), 6);  th = (o + bv) * t
            t1 = t1_p.tile([128, NQ], bf16)
            nc.gpsimd.tensor_scalar(t1, oh, bvs3[:, h:h + 1], 0.0, ALU.add, ALU.max)
            nc.gpsimd.tensor_scalar(t1, t1, 6.0, None, ALU.min)
            obv = t1_p.tile([128, NQ], bf16, tag="obv", name=f"obv_{b}_{h}")
            nc.gpsimd.tensor_scalar(obv, oh, bvs[:, h:h + 1], None, ALU.add)
            nc.gpsimd.tensor_tensor(t2[:, h, b % 2, :], obv, t1, ALU.mult)

        def out_proj(b, t2, qcs=range(5)):
            for qc in qcs:
                r0 = (b - 1) * NQ + qc * 128
                ps = sg_tile()
                for dc in range(8):
                    nc.tensor.matmul(
                        ps[:, 0, :OUT],
                        lhsT=t2[:, dc, :, :].rearrange(
                            "p bb q -> p (bb q)")[:, qc * 128:(qc + 1) * 128],
                        rhs=wp[:, dc, :],
                        start=(dc == 0), stop=(dc == 7),
                    )
                ob = ob_p.tile([128, OUT], f32)
                nc.vector.tensor_tensor(ob, ps[:, 0, :OUT], bps, ALU.add)
                nc.sync.dma_start(out_flat[r0:r0 + 128, :], ob)
                yield

        # prologue: batch 0 inputs + weights; proj(0) up to attention-ready
        xt0, xst0, x8h0, x8l0 = dma_x(0, first=True)
        nc.sync.dma_start(identp, a["identp"])
        nc.sync.dma_start(wv8h, a["wv8h"].rearrange("c p j -> p c j"))
        nc.sync.dma_start(bt8s[0], a["bt8"][0])
        nc.sync.dma_start(wv8l, a["wv8l"].rearrange("c p j -> p c j"))
        nc.sync.dma_start(bt8s[1], a["bt8"][1])
        nc.sync.dma_start(ones, a["ones"])
        nc.sync.dma_start(bvs, a["bv"])
        nc.sync.dma_start(bvs3, a["bv3"])

        # warm up the PE p-state during the input-DMA wait: dummy matmuls
        # on a memset scratch tile so the ramp starts at ~0.5us, reaching
        # full clock before the first real projection matmul
        warm = sg_tile()
        for i in range(8):
            nc.tensor.matmul(
                warm[:, 0, :256],
                lhsT=wk[:, 0, 0:128],
                rhs=wk[:, :, 0:128],
                start=True, stop=True,
            )

        kt, vt, qt, gen0 = proj_gen(0, xt0, xst0, x8h0, x8l0)
        for _ in gen0:          # batch 0 proj must fully precede its attention
            pass
        pending = []
        t2 = None
        nxt = None
        for b in range(BPC):
            if b % 2 == 0:
                t2 = hs_p.tile([128, H, 2, NQ], bf16, tag="t2", name=f"t2_{b}")
            # interleave remaining proj tiles (this batch's tail + next batch)
            if b + 1 < BPC:
                xtn, xstn, x8hn, x8ln = dma_x(b + 1, stagger=(b == 0))
                nxt = proj_gen(b + 1, xtn, xstn, x8hn, x8ln)
                pending.append(nxt[3])
            for h in range(H):
                for gi, _ in enumerate(attention(b, h, kt, vt, qt, t2)):
                    if gi in (1, 2, 3, 4, 5):
                        while pending:
                            if next(pending[0], "done") == "done":
                                pending.pop(0)
                            else:
                                break
            while pending:
                if next(pending[0], "done") == "done":
                    pending.pop(0)
                else:
                    break
            if pending:
                for _ in pending[0]:
                    pass
                pending.pop(0)
            if b % 2 == 1:
                if b + 1 < BPC:
                    pending.append(out_proj(b, t2))  # interleave with next batch
                else:
                    for _ in out_proj(b, t2):
                        pass
            if nxt is not None:
                kt, vt, qt = nxt[0], nxt[1], nxt[2]
                nxt = None


def build():
    import concourse.mybir as mybir
    import concourse.tile as tile
    from concourse import bacc

    nc = bacc.Bacc("TRN2", target_bir_lowering=False, debug=False)
    f32, bf16 = mybir.dt.float32, mybir.dt.bfloat16
    f8e4 = mybir.dt.float8e4
    a = {}

    def din(name, shape, dt=f32):
        a[name] = nc.dram_tensor(name, shape, dt, kind="ExternalInput").ap()

    f32r = mybir.dt.float32r
    din("xt", [BPC, 2, 128, N], f32r)
    din("xst", [BPC, 2, 128, NQ], f32r)
    din("wkt", [2, 128, 512], f32r)
    din("wvt", [2, 128, 1024], f32r)
    din("x8h", [BPC, 2, 128, N], f8e4)
    din("x8l", [BPC, 2, 128, N], f8e4)
    din("wv8h", [2, 128, 1024], f8e4)
    din("wv8l", [2, 128, 1024], f8e4)
    din("wqt", [2, 128, 512], f32r)
    din("wpt", [8, 128, OUT], bf16)
    din("bq", [128, 4])
    din("bv", [128, H])
    din("bv3", [128, H])
    din("bps", [128, OUT])
    din("bt8", [H, 128, NCH, NQ], f8e4)
    din("ones", [128, 128], bf16)
    din("identp", [128, 2, 2, 128], f8e4)
    out_ap = nc.dram_tensor("out", [BPC, NQ, OUT], f32, kind="ExternalOutput").ap()

    with tile.TileContext(nc) as tc:
        _body(tc, a, out_ap)
    nc.compile()
    return nc


_NC_CACHE = None


def _get_nc():
    global _NC_CACHE
    if _NC_CACHE is None:
        _NC_CACHE = build()
    return _NC_CACHE


def kernel(**inputs):
    from concourse.bass_utils import run_bass_kernel_spmd

    in_maps = _prep(inputs)
    nc = _get_nc()
    res = run_bass_kernel_spmd(nc, in_maps, list(range(NCORES)))
    out = np.concatenate([res.results[i]["out"] for i in range(NCORES)], axis=0)
    return np.ascontiguousarray(out, dtype=np.float32)


if __name__ == "__main__":
    rng = np.random.default_rng(0)
    print("smoke: building bass module...")
    nc = build()
    print("built ok:", sum(len(bb.instructions) for bb in nc.m.functions[0].blocks), "instructions")


# revision 11
# speedup vs baseline: 1.3300x; 1.0042x over previous
"""AttentionSubsample Trainium2 kernel.

Full (unsharded) inputs in, full output out. Data-parallel over batch:
32 batches -> 8 NeuronCores x 4 batches each. Weights/biases replicated.

Engine-balance design (cost-model 207.2us/core, vs 275.6us v1 baseline):
  - k-channel BN bias dropped entirely: softmax over n is invariant to
    per-q shifts and (k+bk)@q shifts every key n equally.
  - score bias added pre-exp on the PE as fp8(e4m3) DoubleRow identity
    matmuls (0.5 cyc/row): lhsT=(I,0)/(0,I) selects one chunk of an
    adjacent bias-chunk pair, so the bias stays resident in SBUF stored
    once (3.2KB/partition/head, loaded one time, no per-batch DMA).
  - v projection as fp8 hi/lo split (x = x8h + x8l, Wv = w8h + w8l) with
    three K=256 DoubleRow passes per psum tile, dropping the lo*lo term:
    ~2.7x fewer PE cycles than f32r at bf16-level accuracy. kT/q stay
    f32r: their quantization noise would amplify through exp by sqrt(d).
  - softmax sums: e-tiles accumulated on DVE (bf16 2x mode, in-place
    chain) + one ones-matmul per head instead of 10 PE ones-matmuls.
  - hswish on Pool/DVE: t = min(Relu(o+3+bv), 6) via Pool tensor_scalar
    ops, th = (o+bv)*t via Pool scalar_tensor_tensor; normalize mult on
    Pool; bv folds out of attn@v (softmax rows sum to 1).
  - psum->sbuf copies split across ACT/DVE (GPSIMD cannot touch PSUM on
    real hw); out-proj bias fused into the DVE psum->sbuf add.
  - software pipelining: batch b+1's kT/q/v projection psum tiles are
    emitted interleaved between batch b's attention score groups (one
    tile after every group, via generators), and the pair output
    projection interleaves with the following batch, keeping the PE fed
    through the shared psum-pool rotation.
  - PSUM: scores pool 3x[128,2,512] (chunk pairs at bank-aligned 512
    offsets, exp reads the [*, :320] pair in one ACT instr), po + psm
    1 bank each = 8 banks.
"""

import sys

if "/opt/trn_rl_repo" not in sys.path:
    sys.path.insert(0, "/opt/trn_rl_repo")

import ml_dtypes
import numpy as np

# --- problem constants (hardcoded, must match the grading reference) ---
B, N, C = 32, 1280, 256
H, KD, D = 8, 64, 128          # heads, key dim, value dim per head
NQ = 320                       # subsampled sequence length
OUT = 384
NCORES = 8
BPC = B // NCORES              # batches per core
EPS = 1e-5
NCH = N // 128                 # 10 n-chunks of 128
GRP = 2                        # scores psum group size (n-chunks per group)

# per-head engine tuning: bias add on PE (fp8 DoubleRow) vs DVE (exp-bias mult)
BIAS_PE = [True] * 8
# per-head: softmax sums via 10 PE ones-matmuls vs DVE accumulate + 1 matmul
SUMS_PE = [False] * 8

_PE_HEADS = [h for h in range(H) if BIAS_PE[h]]
_DVE_HEADS = [h for h in range(H) if not BIAS_PE[h]]
_PE_SLOT = {h: i for i, h in enumerate(_PE_HEADS)}
_DVE_SLOT = {h: i for i, h in enumerate(_DVE_HEADS)}

_SUB_IDX = np.concatenate([
    (np.arange(32)[::2][:, None] * 32 + np.arange(32)[::2][None, :]).reshape(-1),
    1024 + (np.arange(16)[::2][:, None] * 16 + np.arange(16)[::2][None, :]).reshape(-1),
])  # [320] subsample row gather


def _prep(inputs):
    """Host-side: fold BN into weights, reorder channels, shard over cores."""
    f32 = np.float32
    x = np.asarray(inputs["x"], f32)
    g_kv, b_kv = np.asarray(inputs["g_kv"], f32), np.asarray(inputs["b_kv"], f32)
    rm_kv, rv_kv = np.asarray(inputs["rm_kv"], f32), np.asarray(inputs["rv_kv"], f32)
    g_q, b_q = np.asarray(inputs["g_q"], f32), np.asarray(inputs["b_q"], f32)
    rm_q, rv_q = np.asarray(inputs["rm_q"], f32), np.asarray(inputs["rv_q"], f32)
    g_p, b_p = np.asarray(inputs["g_p"], f32), np.asarray(inputs["b_p"], f32)
    rm_p, rv_p = np.asarray(inputs["rm_p"], f32), np.asarray(inputs["rv_p"], f32)
    W_kv = np.asarray(inputs["W_kv"], f32)
    W_q = np.asarray(inputs["W_q"], f32)
    W_p = np.asarray(inputs["W_p"], f32)
    attn_bias = np.asarray(inputs["attn_bias"], f32)
    bias_idxs = np.asarray(inputs["bias_idxs"])

    s_kv = g_kv / np.sqrt(rv_kv + EPS)
    Wkv_f = W_kv * s_kv[:, None]
    bkv_f = b_kv - rm_kv * s_kv
    kidx = np.concatenate([np.arange(h * 192, h * 192 + KD) for h in range(H)])
    vidx = np.concatenate([np.arange(h * 192 + KD, (h + 1) * 192) for h in range(H)])
    wkt = np.ascontiguousarray(Wkv_f[kidx].T).reshape(2, 128, 512)     # [c,128][512 kch]
    wvt = np.ascontiguousarray(Wkv_f[vidx].T).reshape(2, 128, 1024)
    bvd = np.ascontiguousarray(bkv_f[vidx].reshape(8, 128).T)          # [128, H]

    scale = KD ** -0.5
    s_q = g_q / np.sqrt(rv_q + EPS)
    wqt = np.ascontiguousarray((W_q * (s_q * scale)[:, None]).T).reshape(2, 128, 512)
    bq = np.ascontiguousarray(((b_q - rm_q * s_q) * scale).reshape(4, 128).T)

    s_p = g_p / np.sqrt(rv_p + EPS)
    wpt = np.ascontiguousarray((W_p * s_p[:, None]).T / 6.0).reshape(
        8, 128, OUT).astype(ml_dtypes.bfloat16)
    bps = np.ascontiguousarray(np.broadcast_to(b_p - rm_p * s_p, (128, OUT))).astype(np.float32)

    biasT = attn_bias[:, bias_idxs].transpose(0, 2, 1)                 # [H, N, NQ]
    bias_cpq = biasT.reshape(H, NCH, 128, NQ).transpose(0, 2, 1, 3)    # [H,128,NCH,NQ]
    f8 = ml_dtypes.float8_e4m3
    # bias fp8, stored once per head; the DoubleRow identity pair (I,0)/(0,I)
    # selects one chunk of an adjacent pair per instruction
    bt8 = np.ascontiguousarray(bias_cpq).astype(f8)                    # [H,128,NCH,NQ]

    identp = np.zeros((128, 2, 2, 128), f8)
    identp[np.arange(128), 0, 0, np.arange(128)] = 1.0
    identp[np.arange(128), 1, 1, np.arange(128)] = 1.0

    wv8h = wvt.astype(f8)
    wv8l = (wvt - wv8h.astype(np.float32)).astype(f8)

    xs = x[:, _SUB_IDX, :]                                             # [B, NQ, C]
    in_maps = []
    for i in range(NCORES):
        sl = slice(i * BPC, (i + 1) * BPC)
        xt = np.ascontiguousarray(x[sl].transpose(0, 2, 1)).reshape(BPC, 2, 128, N)
        x8h = xt.astype(f8)
        x8l = (xt - x8h.astype(np.float32)).astype(f8)
        xst = np.ascontiguousarray(xs[sl].transpose(0, 2, 1)).reshape(BPC, 2, 128, NQ)
        in_maps.append({
            "xt": xt, "xst": xst, "x8h": x8h, "x8l": x8l,
            "wv8h": wv8h, "wv8l": wv8l,
            "wkt": wkt, "wvt": wvt, "wqt": wqt, "wpt": wpt,
            "bq": bq, "bv": bvd, "bv3": bvd + 3.0, "bps": bps,
            "bt8": bt8,
            "ones": np.ones((128, 128), ml_dtypes.bfloat16),
            "identp": identp,
        })
    return in_maps


def _body(tc, a, out_ap):
    import concourse.bass as bass  # noqa: F401
    import concourse.mybir as mybir
    from contextlib import ExitStack

    nc = tc.nc
    f32 = mybir.dt.float32
    f32r = mybir.dt.float32r
    bf16 = mybir.dt.bfloat16
    f8e4 = mybir.dt.float8e4
    AF = mybir.ActivationFunctionType
    ALU = mybir.AluOpType
    PM = mybir.MatmulPerfMode

    with ExitStack() as ctx:
        ctx.enter_context(
            nc.allow_low_precision(reason="bf16 o-side + fp8 bias matmuls are deliberate; verified vs fp32 reference")
        )
        singles = ctx.enter_context(tc.tile_pool(name="singles", bufs=1))
        # DMA order matters at startup: first-needed weights first (wk -> q/kT
        # projections of batch 0), small attention-phase tiles later.
        wk = singles.tile([128, 2, 512], f32r)
        nc.sync.dma_start(wk[:, :, 0:128], a["wkt"][:, :, 0:128].rearrange("c p j -> p c j"))
        wq = singles.tile([128, 2, 512], f32r)
        bqs = singles.tile([128, 4], f32)
        wv8h = singles.tile([128, 2, 1024], f8e4)
        wv8l = singles.tile([128, 2, 1024], f8e4)
        wp = singles.tile([128, 8, OUT], bf16)
        bvs = singles.tile([128, H], f32)
        bvs3 = singles.tile([128, H], f32)
        ones = singles.tile([128, 128], bf16)
        identp = singles.tile([128, 2, 2, 128], f8e4)
        bps = singles.tile([128, OUT], f32)
        bt8s = [singles.tile([128, NCH, NQ], f8e4, name=f"bt8h{h}")
                for h in range(H)]

        xt_p = ctx.enter_context(tc.tile_pool(name="xt", bufs=1))
        xst_p = ctx.enter_context(tc.tile_pool(name="xst", bufs=1))
        x8_p = ctx.enter_context(tc.tile_pool(name="x8", bufs=1))
        kt_p = ctx.enter_context(tc.tile_pool(name="kt", bufs=2))
        v_p = ctx.enter_context(tc.tile_pool(name="v", bufs=2))
        qt_p = ctx.enter_context(tc.tile_pool(name="qt", bufs=3))
        e_p = ctx.enter_context(tc.tile_pool(name="e", bufs=8))
        esum_p = ctx.enter_context(tc.tile_pool(name="esum", bufs=2))
        orw_p = ctx.enter_context(tc.tile_pool(name="orw", bufs=3))
        rc_p = ctx.enter_context(tc.tile_pool(name="rc", bufs=3))
        oh_p = ctx.enter_context(tc.tile_pool(name="oh", bufs=3))
        t1_p = ctx.enter_context(tc.tile_pool(name="t1", bufs=3))
        hs_p = ctx.enter_context(tc.tile_pool(name="hs", bufs=2))
        ob_p = ctx.enter_context(tc.tile_pool(name="ob", bufs=3))
        ps_sg = ctx.enter_context(tc.tile_pool(name="ps_sg", bufs=3, space="PSUM"))
        ps_o = ctx.enter_context(tc.tile_pool(name="ps_o", bufs=1, space="PSUM"))
        ps_sum = ctx.enter_context(tc.tile_pool(name="ps_sum", bufs=1, space="PSUM"))

        _wt_n = [0]

        def sg_tile():
            _wt_n[0] += 1
            return ps_sg.tile([128, GRP, 512], f32, tag="sg", name=f"sg{_wt_n[0]}")

        out_flat = out_ap.rearrange("b q o -> (b q) o")

        def dma_x(b, first=False, stagger=False):
            """Issue input DMAs for batch b; returns (xt, xst) tiles."""
            xt = xt_p.tile([128, 2, N], f32r, tag="xt", name=f"xt{b}")
            for ns in range(3):
                if stagger:        # keep the resident-bias stream fed first
                    nc.sync.dma_start(bt8s[2 + ns], a["bt8"][2 + ns])
                n0 = ns * 512
                nsz = min(512, N - n0)
                nc.sync.dma_start(
                    xt[:, :, n0:n0 + nsz],
                    a["xt"][b, :, :, n0:n0 + nsz].rearrange("c p n -> p c n"),
                )
                if first and ns == 0:
                    nc.sync.dma_start(
                        wk[:, :, 128:512],
                        a["wkt"][:, :, 128:512].rearrange("c p j -> p c j"))
                if first and ns == 1:
                    nc.sync.dma_start(wq, a["wqt"].rearrange("c p j -> p c j"))
                    nc.sync.dma_start(bqs, a["bq"])
            if stagger:
                nc.sync.dma_start(bt8s[5], a["bt8"][5])
            xst = xst_p.tile([128, 2, NQ], f32r, tag="xst", name=f"xst{b}")
            nc.sync.dma_start(xst, a["xst"][b].rearrange("c p n -> p c n"))
            x8h = x8_p.tile([128, 2, N], f8e4, tag="x8h", name=f"x8h{b}")
            nc.sync.dma_start(x8h, a["x8h"][b].rearrange("c p n -> p c n"))
            if stagger:
                nc.sync.dma_start(bt8s[6], a["bt8"][6])
            x8l = x8_p.tile([128, 2, N], f8e4, tag="x8l", name=f"x8l{b}")
            nc.sync.dma_start(x8l, a["x8l"][b].rearrange("c p n -> p c n"))
            if stagger:
                nc.sync.dma_start(bt8s[7], a["bt8"][7])
                nc.sync.dma_start(wp, a["wpt"].rearrange("c p j -> p c j"))
                nc.sync.dma_start(bps, a["bps"])
            return xt, xst, x8h, x8l

        def proj_gen(b, xt, xst, x8h, x8l):
            """Yield after each proj psum tile; returns (kt, vt, qt) eagerly."""
            kt = kt_p.tile([128, 4, N], f32r, tag="kt", name=f"kt{b}")
            vt = v_p.tile([128, NCH, 1024], bf16, tag="vt", name=f"vt{b}")
            qt = qt_p.tile([128, 4, NQ], f32r, tag="qt", name=f"qt{b}")

            def emit():
                # kT projection: no bias (softmax-invariant), wide copies
                for pr in range(4):
                    ps = sg_tile()
                    for half in range(2):       # n slices 0:512, 512:1024
                        n0 = half * 512
                        for cc in range(2):
                            nc.tensor.matmul(
                                ps[:, half, :],
                                lhsT=wk[:, cc, pr * 128:(pr + 1) * 128],
                                rhs=xt[:, cc, n0:n0 + 512],
                                start=(cc == 0), stop=(cc == 1),
                            )
                    if pr < 2:
                        nc.vector.tensor_copy(
                            kt[:, pr, 0:1024], ps.rearrange("p g j -> p (g j)"),
                        )
                    else:
                        nc.scalar.copy(
                            kt[:, pr, 0:1024], ps.rearrange("p g j -> p (g j)"),
                        )
                    yield
                ps = sg_tile()                  # 256-col tails, two prs per tile
                for prh in range(2):
                    for j in range(2):
                        pr = 2 * prh + j
                        for cc in range(2):
                            nc.tensor.matmul(
                                ps[:, j, :256] if prh == 0 else ps[:, j, 256:512],
                                lhsT=wk[:, cc, pr * 128:(pr + 1) * 128],
                                rhs=xt[:, cc, 1024:N],
                                start=(cc == 0), stop=(cc == 1),
                            )
                        if prh == 0:
                            nc.scalar.copy(kt[:, pr, 1024:N], ps[:, j, :256])
                        else:
                            nc.scalar.copy(kt[:, pr, 1024:N], ps[:, j, 256:512])
                yield
                # q projection: 2 prs per tile, fused bias on DVE
                for half in range(2):
                    ps = sg_tile()
                    for j in range(2):
                        pr = 2 * half + j
                        for cc in range(2):
                            nc.tensor.matmul(
                                ps[:, j, :NQ],
                                lhsT=wq[:, cc, pr * 128:(pr + 1) * 128],
                                rhs=xst[:, cc, :],
                                start=(cc == 0), stop=(cc == 1),
                            )
                    nc.vector.tensor_tensor(
                        qt[:, 2 * half:2 * half + 2, :], ps[:, :, :NQ],
                        bqs[:, 2 * half:2 * half + 2].to_broadcast((128, 2, NQ)),
                        ALU.add,
                    )
                    yield
                # v projection: fp8 hi/lo DoubleRow (K=256 per pass, 3 passes)
                for cn in range(NCH):
                    ps = sg_tile()
                    for hf in range(2):
                        for pi, (xx, ww) in enumerate(
                                ((x8h, wv8h), (x8l, wv8h), (x8h, wv8l))):
                            nc.tensor.matmul(
                                ps[:, hf, :],
                                lhsT=xx[:, :, cn * 128:(cn + 1) * 128],
                                rhs=ww[:, :, hf * 512:(hf + 1) * 512],
                                start=(pi == 0), stop=(pi == 2),
                                perf_mode=PM.DoubleRow,
                            )
                    if cn % 2 == 0:
                        nc.vector.tensor_copy(
                            vt[:, cn, :], ps.rearrange("p g j -> p (g j)"))
                    else:
                        nc.scalar.copy(
                            vt[:, cn, :], ps.rearrange("p g j -> p (g j)"))
                    yield

            return kt, vt, qt, emit()

        def attention(b, h, kt, vt, qt, t2):
            # generator: yields after each score group so the driver can
            # weave projection tiles of the next batch between groups
            yield
            pr, p0 = h // 2, 64 * (h % 2)
            on_pe = BIAS_PE[h]
            bt8 = bt8s[h]
            po = ps_o.tile([128, NQ], f32, tag="po", name=f"po_{b}_{h}")
            e_tiles = []
            for g in range(NCH // GRP):
                sg = sg_tile()
                for j in range(GRP):
                    c = GRP * g + j
                    nc.tensor.matmul(
                        sg[:, j, :NQ],
                        lhsT=kt[p0:p0 + 64, pr, c * 128:(c + 1) * 128],
                        rhs=qt[p0:p0 + 64, pr, :],
                        start=True, stop=(not on_pe),
                    )
                    if on_pe:
                        nc.tensor.matmul(
                            sg[:, j, :NQ],
                            lhsT=identp[:, j, :, :],
                            rhs=bt8[:, GRP * g:GRP * (g + 1), :],
                            start=False, stop=True,
                            perf_mode=PM.DoubleRow,
                        )
                e = e_p.tile([128, GRP, NQ], bf16)
                nc.scalar.activation(e, sg[:, :, :NQ], AF.Exp)
                e_tiles.append(e)
                for j in range(GRP):
                    c = GRP * g + j
                    nc.tensor.matmul(
                        po[:, :NQ],
                        lhsT=vt[:, c, h * 128:(h + 1) * 128],
                        rhs=e[:, j, :],
                        start=(c == 0), stop=(c == NCH - 1),
                    )
                yield
            # softmax denominators
            psm = ps_sum.tile([128, NQ], f32, tag="psm", name=f"psm_{b}_{h}")
            if SUMS_PE[h]:
                for g in range(NCH // GRP):
                    for j in range(GRP):
                        c = GRP * g + j
                        nc.tensor.matmul(
                            psm,
                            lhsT=ones,
                            rhs=e_tiles[g][:, j, :],
                            start=(c == 0), stop=(c == NCH - 1),
                        )
            else:
                t12 = e_tiles[1]
                nc.vector.tensor_tensor(t12, e_tiles[0], e_tiles[1], ALU.add)
                t34 = e_tiles[3]
                nc.vector.tensor_tensor(t34, e_tiles[2], e_tiles[3], ALU.add)
                nc.vector.tensor_tensor(t12, t12, t34, ALU.add)
                nc.vector.tensor_tensor(t12, t12, e_tiles[4], ALU.add)
                esum = esum_p.tile([128, NQ], bf16)
                nc.vector.tensor_tensor(esum, t12[:, 0, :], t12[:, 1, :], ALU.add)
                nc.tensor.matmul(psm, lhsT=ones, rhs=esum, start=True, stop=True)
            rc = rc_p.tile([128, NQ], bf16)
            nc.vector.reciprocal(rc, psm)
            oraw = orw_p.tile([128, NQ], bf16, tag="oraw", name=f"oraw_{b}_{h}")
            nc.vector.tensor_copy(oraw, po)
            oh = oh_p.tile([128, NQ], bf16)
            nc.gpsimd.tensor_tensor(oh, oraw, rc, ALU.mult)
            # hswish: t = min(Relu(o + 3 + bv), 6);  th = (o + bv) * t
            t1 = t1_p.tile([128, NQ], bf16)
            nc.gpsimd.tensor_scalar(t1, oh, bvs3[:, h:h + 1], 0.0, ALU.add, ALU.max)
            nc.gpsimd.tensor_scalar(t1, t1, 6.0, None, ALU.min)
            obv = t1_p.tile([128, NQ], bf16, tag="obv", name=f"obv_{b}_{h}")
            nc.gpsimd.tensor_scalar(obv, oh, bvs[:, h:h + 1], None, ALU.add)
            nc.gpsimd.tensor_tensor(t2[:, h, b % 2, :], obv, t1, ALU.mult)

        def out_proj(b, t2, qcs=range(5)):
            for qc in qcs:
                r0 = (b - 1) * NQ + qc * 128
                ps = sg_tile()
                for dc in range(8):
                    nc.tensor.matmul(
                        ps[:, 0, :OUT],
                        lhsT=t2[:, dc, :, :].rearrange(
                            "p bb q -> p (bb q)")[:, qc * 128:(qc + 1) * 128],
                        rhs=wp[:, dc, :],
                        start=(dc == 0), stop=(dc == 7),
                    )
                ob = ob_p.tile([128, OUT], f32)
                nc.vector.tensor_tensor(ob, ps[:, 0, :OUT], bps, ALU.add)
                nc.sync.dma_start(out_flat[r0:r0 + 128, :], ob)
                yield

        # prologue: batch 0 inputs + weights; proj(0) up to attention-ready
        xt0, xst0, x8h0, x8l0 = dma_x(0, first=True)
        nc.sync.dma_start(identp, a["identp"])
        nc.sync.dma_start(wv8h, a["wv8h"].rearrange("c p j -> p c j"))
        nc.sync.dma_start(bt8s[0], a["bt8"][0])
        nc.sync.dma_start(wv8l, a["wv8l"].rearrange("c p j -> p c j"))
        nc.sync.dma_start(bt8s[1], a["bt8"][1])
        nc.sync.dma_start(ones, a["ones"])
        nc.sync.dma_start(bvs, a["bv"])
        nc.sync.dma_start(bvs3, a["bv3"])

        # warm up the PE p-state during the input-DMA wait: dummy matmuls
        # on a memset scratch tile so the ramp starts at ~0.5us, reaching
        # full clock before the first real projection matmul
        warm = sg_tile()
        for i in range(8):
            nc.tensor.matmul(
                warm[:, 0, :256],
                lhsT=wk[:, 0, 0:128],
                rhs=wk[:, :, 0:128],
                start=True, stop=True,
            )

        kt, vt, qt, gen0 = proj_gen(0, xt0, xst0, x8h0, x8l0)
        for _ in gen0:          # batch 0 proj must fully precede its attention
            pass
        pending = []
        t2 = None
        nxt = None
        for b in range(BPC):
            if b % 2 == 0:
                t2 = hs_p.tile([128, H, 2, NQ], bf16, tag="t2", name=f"t2_{b}")
            # interleave remaining proj tiles (this batch's tail + next batch)
            if b + 1 < BPC:
                xtn, xstn, x8hn, x8ln = dma_x(b + 1, stagger=(b == 0))
                nxt = proj_gen(b + 1, xtn, xstn, x8hn, x8ln)
                pending.append(nxt[3])
            for h in range(H):
                for gi, _ in enumerate(attention(b, h, kt, vt, qt, t2)):
                    for _ in range(2 if gi == 5 else (1 if gi else 0)):
                        while pending:
                            if next(pending[0], "done") == "done":
                                pending.pop(0)
                            else:
                                break
            while pending:
                if next(pending[0], "done") == "done":
                    pending.pop(0)
                else:
                    break
            if pending:
                for _ in pending[0]:
                    pass
                pending.pop(0)
            if b % 2 == 1:
                if b + 1 < BPC:
                    pending.append(out_proj(b, t2))  # interleave with next batch
                else:
                    for _ in out_proj(b, t2):
                        pass
            if nxt is not None:
                kt, vt, qt = nxt[0], nxt[1], nxt[2]
                nxt = None


def build():
    import concourse.mybir as mybir
    import concourse.tile as tile
    from concourse import bacc

    nc = bacc.Bacc("TRN2", target_bir_lowering=False, debug=False)
    f32, bf16 = mybir.dt.float32, mybir.dt.bfloat16
    f8e4 = mybir.dt.float8e4
    a = {}

    def din(name, shape, dt=f32):
        a[name] = nc.dram_tensor(name, shape, dt, kind="ExternalInput").ap()

    f32r = mybir.dt.float32r
    din("xt", [BPC, 2, 128, N], f32r)
    din("xst", [BPC, 2, 128, NQ], f32r)
    din("wkt", [2, 128, 512], f32r)
    din("wvt", [2, 128, 1024], f32r)
    din("x8h", [BPC, 2, 128, N], f8e4)
    din("x8l", [BPC, 2, 128, N], f8e4)
    din("wv8h", [2, 128, 1024], f8e4)
    din("wv8l", [2, 128, 1024], f8e4)
    din("wqt", [2, 128, 512], f32r)
    din("wpt", [8, 128, OUT], bf16)
    din("bq", [128, 4])
    din("bv", [128, H])
    din("bv3", [128, H])
    din("bps", [128, OUT])
    din("bt8", [H, 128, NCH, NQ], f8e4)
    din("ones", [128, 128], bf16)
    din("identp", [128, 2, 2, 128], f8e4)
    out_ap = nc.dram_tensor("out", [BPC, NQ, OUT], f32, kind="ExternalOutput").ap()

    with tile.TileContext(nc) as tc:
        _body(tc, a, out_ap)
    nc.compile()
    return nc


_NC_CACHE = None


def _get_nc():
    global _NC_CACHE
    if _NC_CACHE is None:
        _NC_CACHE = build()
    return _NC_CACHE


def kernel(**inputs):
    from concourse.bass_utils import run_bass_kernel_spmd

    in_maps = _prep(inputs)
    nc = _get_nc()
    res = run_bass_kernel_spmd(nc, in_maps, list(range(NCORES)))
    out = np.concatenate([res.results[i]["out"] for i in range(NCORES)], axis=0)
    return np.ascontiguousarray(out, dtype=np.float32)


if __name__ == "__main__":
    rng = np.random.default_rng(0)
    print("smoke: building bass module...")
    nc = build()
    print("built ok:", sum(len(bb.instructions) for bb in nc.m.functions[0].blocks), "instructions")


# revision 12
# speedup vs baseline: 1.3323x; 1.0017x over previous
"""AttentionSubsample Trainium2 kernel.

Full (unsharded) inputs in, full output out. Data-parallel over batch:
32 batches -> 8 NeuronCores x 4 batches each. Weights/biases replicated.

Engine-balance design (cost-model 206.9us/core, vs 275.6us v1 baseline):
  - k-channel BN bias dropped entirely: softmax over n is invariant to
    per-q shifts and (k+bk)@q shifts every key n equally.
  - score bias added pre-exp on the PE as fp8(e4m3) DoubleRow identity
    matmuls (0.5 cyc/row): lhsT=(I,0)/(0,I) selects one chunk of an
    adjacent bias-chunk pair, so the bias stays resident in SBUF stored
    once (3.2KB/partition/head, loaded one time, no per-batch DMA).
  - v projection as fp8 hi/lo split (x = x8h + x8l, Wv = w8h + w8l) with
    three K=256 DoubleRow passes per psum tile, dropping the lo*lo term:
    ~2.7x fewer PE cycles than f32r at bf16-level accuracy. kT/q stay
    f32r: their quantization noise would amplify through exp by sqrt(d).
  - softmax sums: e-tiles accumulated on DVE (bf16 2x mode, in-place
    chain) + one ones-matmul per head instead of 10 PE ones-matmuls.
  - hswish on Pool/DVE: t = min(Relu(o+3+bv), 6) via Pool tensor_scalar
    ops, th = (o+bv)*t via Pool scalar_tensor_tensor; normalize mult on
    Pool; bv folds out of attn@v (softmax rows sum to 1).
  - psum->sbuf copies split across ACT/DVE (GPSIMD cannot touch PSUM on
    real hw); out-proj bias fused into the DVE psum->sbuf add.
  - software pipelining: batch b+1's kT/q/v projection psum tiles are
    emitted interleaved between batch b's attention score groups (one
    tile after every group, via generators), and the pair output
    projection interleaves with the following batch, keeping the PE fed
    through the shared psum-pool rotation.
  - PSUM: scores pool 3x[128,2,512] (chunk pairs at bank-aligned 512
    offsets, exp reads the [*, :320] pair in one ACT instr), po + psm
    1 bank each = 8 banks.
"""

import sys

if "/opt/trn_rl_repo" not in sys.path:
    sys.path.insert(0, "/opt/trn_rl_repo")

import ml_dtypes
import numpy as np

# --- problem constants (hardcoded, must match the grading reference) ---
B, N, C = 32, 1280, 256
H, KD, D = 8, 64, 128          # heads, key dim, value dim per head
NQ = 320                       # subsampled sequence length
OUT = 384
NCORES = 8
BPC = B // NCORES              # batches per core
EPS = 1e-5
NCH = N // 128                 # 10 n-chunks of 128
GRP = 2                        # scores psum group size (n-chunks per group)

# per-head engine tuning: bias add on PE (fp8 DoubleRow) vs DVE (exp-bias mult)
BIAS_PE = [True] * 8
# per-head: softmax sums via 10 PE ones-matmuls vs DVE accumulate + 1 matmul
SUMS_PE = [False] * 8

_PE_HEADS = [h for h in range(H) if BIAS_PE[h]]
_DVE_HEADS = [h for h in range(H) if not BIAS_PE[h]]
_PE_SLOT = {h: i for i, h in enumerate(_PE_HEADS)}
_DVE_SLOT = {h: i for i, h in enumerate(_DVE_HEADS)}

_SUB_IDX = np.concatenate([
    (np.arange(32)[::2][:, None] * 32 + np.arange(32)[::2][None, :]).reshape(-1),
    1024 + (np.arange(16)[::2][:, None] * 16 + np.arange(16)[::2][None, :]).reshape(-1),
])  # [320] subsample row gather


def _prep(inputs):
    """Host-side: fold BN into weights, reorder channels, shard over cores."""
    f32 = np.float32
    x = np.asarray(inputs["x"], f32)
    g_kv, b_kv = np.asarray(inputs["g_kv"], f32), np.asarray(inputs["b_kv"], f32)
    rm_kv, rv_kv = np.asarray(inputs["rm_kv"], f32), np.asarray(inputs["rv_kv"], f32)
    g_q, b_q = np.asarray(inputs["g_q"], f32), np.asarray(inputs["b_q"], f32)
    rm_q, rv_q = np.asarray(inputs["rm_q"], f32), np.asarray(inputs["rv_q"], f32)
    g_p, b_p = np.asarray(inputs["g_p"], f32), np.asarray(inputs["b_p"], f32)
    rm_p, rv_p = np.asarray(inputs["rm_p"], f32), np.asarray(inputs["rv_p"], f32)
    W_kv = np.asarray(inputs["W_kv"], f32)
    W_q = np.asarray(inputs["W_q"], f32)
    W_p = np.asarray(inputs["W_p"], f32)
    attn_bias = np.asarray(inputs["attn_bias"], f32)
    bias_idxs = np.asarray(inputs["bias_idxs"])

    s_kv = g_kv / np.sqrt(rv_kv + EPS)
    Wkv_f = W_kv * s_kv[:, None]
    bkv_f = b_kv - rm_kv * s_kv
    kidx = np.concatenate([np.arange(h * 192, h * 192 + KD) for h in range(H)])
    vidx = np.concatenate([np.arange(h * 192 + KD, (h + 1) * 192) for h in range(H)])
    wkt = np.ascontiguousarray(Wkv_f[kidx].T).reshape(2, 128, 512)     # [c,128][512 kch]
    wvt = np.ascontiguousarray(Wkv_f[vidx].T).reshape(2, 128, 1024)
    bvd = np.ascontiguousarray(bkv_f[vidx].reshape(8, 128).T)          # [128, H]

    scale = KD ** -0.5
    s_q = g_q / np.sqrt(rv_q + EPS)
    wqt = np.ascontiguousarray((W_q * (s_q * scale)[:, None]).T).reshape(2, 128, 512)
    bq = np.ascontiguousarray(((b_q - rm_q * s_q) * scale).reshape(4, 128).T)

    s_p = g_p / np.sqrt(rv_p + EPS)
    wpt = np.ascontiguousarray((W_p * s_p[:, None]).T / 6.0).reshape(
        8, 128, OUT).astype(ml_dtypes.bfloat16)
    bps = np.ascontiguousarray(np.broadcast_to(b_p - rm_p * s_p, (128, OUT))).astype(np.float32)

    biasT = attn_bias[:, bias_idxs].transpose(0, 2, 1)                 # [H, N, NQ]
    bias_cpq = biasT.reshape(H, NCH, 128, NQ).transpose(0, 2, 1, 3)    # [H,128,NCH,NQ]
    f8 = ml_dtypes.float8_e4m3
    # bias fp8, stored once per head; the DoubleRow identity pair (I,0)/(0,I)
    # selects one chunk of an adjacent pair per instruction
    bt8 = np.ascontiguousarray(bias_cpq).astype(f8)                    # [H,128,NCH,NQ]

    identp = np.zeros((128, 2, 2, 128), f8)
    identp[np.arange(128), 0, 0, np.arange(128)] = 1.0
    identp[np.arange(128), 1, 1, np.arange(128)] = 1.0

    wv8h = wvt.astype(f8)
    wv8l = (wvt - wv8h.astype(np.float32)).astype(f8)

    xs = x[:, _SUB_IDX, :]                                             # [B, NQ, C]
    in_maps = []
    for i in range(NCORES):
        sl = slice(i * BPC, (i + 1) * BPC)
        xt = np.ascontiguousarray(x[sl].transpose(0, 2, 1)).reshape(BPC, 2, 128, N)
        x8h = xt.astype(f8)
        x8l = (xt - x8h.astype(np.float32)).astype(f8)
        xst = np.ascontiguousarray(xs[sl].transpose(0, 2, 1)).reshape(BPC, 2, 128, NQ)
        in_maps.append({
            "xt": xt, "xst": xst, "x8h": x8h, "x8l": x8l,
            "wv8h": wv8h, "wv8l": wv8l,
            "wkt": wkt, "wvt": wvt, "wqt": wqt, "wpt": wpt,
            "bq": bq, "bv": bvd, "bv3": bvd + 3.0, "bps": bps,
            "bt8": bt8,
            "ones": np.ones((128, 128), ml_dtypes.bfloat16),
            "identp": identp,
        })
    return in_maps


def _body(tc, a, out_ap):
    import concourse.bass as bass  # noqa: F401
    import concourse.mybir as mybir
    from contextlib import ExitStack

    nc = tc.nc
    f32 = mybir.dt.float32
    f32r = mybir.dt.float32r
    bf16 = mybir.dt.bfloat16
    f8e4 = mybir.dt.float8e4
    AF = mybir.ActivationFunctionType
    ALU = mybir.AluOpType
    PM = mybir.MatmulPerfMode

    with ExitStack() as ctx:
        ctx.enter_context(
            nc.allow_low_precision(reason="bf16 o-side + fp8 bias matmuls are deliberate; verified vs fp32 reference")
        )
        singles = ctx.enter_context(tc.tile_pool(name="singles", bufs=1))
        # DMA order matters at startup: first-needed weights first (wk -> q/kT
        # projections of batch 0), small attention-phase tiles later.
        wk = singles.tile([128, 2, 512], f32r)
        nc.sync.dma_start(wk[:, :, 0:128], a["wkt"][:, :, 0:128].rearrange("c p j -> p c j"))
        wq = singles.tile([128, 2, 512], f32r)
        bqs = singles.tile([128, 4], f32)
        wv8h = singles.tile([128, 2, 1024], f8e4)
        wv8l = singles.tile([128, 2, 1024], f8e4)
        wp = singles.tile([128, 8, OUT], bf16)
        bvs = singles.tile([128, H], f32)
        bvs3 = singles.tile([128, H], f32)
        ones = singles.tile([128, 128], bf16)
        identp = singles.tile([128, 2, 2, 128], f8e4)
        bps = singles.tile([128, OUT], f32)
        bt8s = [singles.tile([128, NCH, NQ], f8e4, name=f"bt8h{h}")
                for h in range(H)]

        xt_p = ctx.enter_context(tc.tile_pool(name="xt", bufs=1))
        xst_p = ctx.enter_context(tc.tile_pool(name="xst", bufs=1))
        x8_p = ctx.enter_context(tc.tile_pool(name="x8", bufs=1))
        kt_p = ctx.enter_context(tc.tile_pool(name="kt", bufs=2))
        v_p = ctx.enter_context(tc.tile_pool(name="v", bufs=2))
        qt_p = ctx.enter_context(tc.tile_pool(name="qt", bufs=3))
        e_p = ctx.enter_context(tc.tile_pool(name="e", bufs=10))
        esum_p = ctx.enter_context(tc.tile_pool(name="esum", bufs=3))
        orw_p = ctx.enter_context(tc.tile_pool(name="orw", bufs=3))
        rc_p = ctx.enter_context(tc.tile_pool(name="rc", bufs=3))
        oh_p = ctx.enter_context(tc.tile_pool(name="oh", bufs=3))
        t1_p = ctx.enter_context(tc.tile_pool(name="t1", bufs=3))
        hs_p = ctx.enter_context(tc.tile_pool(name="hs", bufs=2))
        ob_p = ctx.enter_context(tc.tile_pool(name="ob", bufs=4))
        ps_sg = ctx.enter_context(tc.tile_pool(name="ps_sg", bufs=3, space="PSUM"))
        ps_o = ctx.enter_context(tc.tile_pool(name="ps_o", bufs=1, space="PSUM"))
        ps_sum = ctx.enter_context(tc.tile_pool(name="ps_sum", bufs=1, space="PSUM"))

        _wt_n = [0]

        def sg_tile():
            _wt_n[0] += 1
            return ps_sg.tile([128, GRP, 512], f32, tag="sg", name=f"sg{_wt_n[0]}")

        out_flat = out_ap.rearrange("b q o -> (b q) o")

        def dma_x(b, first=False, stagger=False):
            """Issue input DMAs for batch b; returns (xt, xst) tiles."""
            xt = xt_p.tile([128, 2, N], f32r, tag="xt", name=f"xt{b}")
            for ns in range(3):
                if stagger:        # keep the resident-bias stream fed first
                    nc.sync.dma_start(bt8s[2 + ns], a["bt8"][2 + ns])
                n0 = ns * 512
                nsz = min(512, N - n0)
                nc.sync.dma_start(
                    xt[:, :, n0:n0 + nsz],
                    a["xt"][b, :, :, n0:n0 + nsz].rearrange("c p n -> p c n"),
                )
                if first and ns == 0:
                    nc.sync.dma_start(
                        wk[:, :, 128:512],
                        a["wkt"][:, :, 128:512].rearrange("c p j -> p c j"))
                if first and ns == 1:
                    nc.sync.dma_start(wq, a["wqt"].rearrange("c p j -> p c j"))
                    nc.sync.dma_start(bqs, a["bq"])
            if stagger:
                nc.sync.dma_start(bt8s[5], a["bt8"][5])
            xst = xst_p.tile([128, 2, NQ], f32r, tag="xst", name=f"xst{b}")
            nc.sync.dma_start(xst, a["xst"][b].rearrange("c p n -> p c n"))
            x8h = x8_p.tile([128, 2, N], f8e4, tag="x8h", name=f"x8h{b}")
            nc.sync.dma_start(x8h, a["x8h"][b].rearrange("c p n -> p c n"))
            if stagger:
                nc.sync.dma_start(bt8s[6], a["bt8"][6])
            x8l = x8_p.tile([128, 2, N], f8e4, tag="x8l", name=f"x8l{b}")
            nc.sync.dma_start(x8l, a["x8l"][b].rearrange("c p n -> p c n"))
            if stagger:
                nc.sync.dma_start(bt8s[7], a["bt8"][7])
                nc.sync.dma_start(wp, a["wpt"].rearrange("c p j -> p c j"))
                nc.sync.dma_start(bps, a["bps"])
            return xt, xst, x8h, x8l

        def proj_gen(b, xt, xst, x8h, x8l):
            """Yield after each proj psum tile; returns (kt, vt, qt) eagerly."""
            kt = kt_p.tile([128, 4, N], f32r, tag="kt", name=f"kt{b}")
            vt = v_p.tile([128, NCH, 1024], bf16, tag="vt", name=f"vt{b}")
            qt = qt_p.tile([128, 4, NQ], f32r, tag="qt", name=f"qt{b}")

            def emit():
                # kT projection: no bias (softmax-invariant), wide copies
                for pr in range(4):
                    ps = sg_tile()
                    for half in range(2):       # n slices 0:512, 512:1024
                        n0 = half * 512
                        for cc in range(2):
                            nc.tensor.matmul(
                                ps[:, half, :],
                                lhsT=wk[:, cc, pr * 128:(pr + 1) * 128],
                                rhs=xt[:, cc, n0:n0 + 512],
                                start=(cc == 0), stop=(cc == 1),
                            )
                    if pr < 2:
                        nc.vector.tensor_copy(
                            kt[:, pr, 0:1024], ps.rearrange("p g j -> p (g j)"),
                        )
                    else:
                        nc.scalar.copy(
                            kt[:, pr, 0:1024], ps.rearrange("p g j -> p (g j)"),
                        )
                    yield
                ps = sg_tile()                  # 256-col tails, two prs per tile
                for prh in range(2):
                    for j in range(2):
                        pr = 2 * prh + j
                        for cc in range(2):
                            nc.tensor.matmul(
                                ps[:, j, :256] if prh == 0 else ps[:, j, 256:512],
                                lhsT=wk[:, cc, pr * 128:(pr + 1) * 128],
                                rhs=xt[:, cc, 1024:N],
                                start=(cc == 0), stop=(cc == 1),
                            )
                        if prh == 0:
                            nc.scalar.copy(kt[:, pr, 1024:N], ps[:, j, :256])
                        else:
                            nc.scalar.copy(kt[:, pr, 1024:N], ps[:, j, 256:512])
                yield
                # q projection: 2 prs per tile, fused bias on DVE
                for half in range(2):
                    ps = sg_tile()
                    for j in range(2):
                        pr = 2 * half + j
                        for cc in range(2):
                            nc.tensor.matmul(
                                ps[:, j, :NQ],
                                lhsT=wq[:, cc, pr * 128:(pr + 1) * 128],
                                rhs=xst[:, cc, :],
                                start=(cc == 0), stop=(cc == 1),
                            )
                    nc.vector.tensor_tensor(
                        qt[:, 2 * half:2 * half + 2, :], ps[:, :, :NQ],
                        bqs[:, 2 * half:2 * half + 2].to_broadcast((128, 2, NQ)),
                        ALU.add,
                    )
                    yield
                # v projection: fp8 hi/lo DoubleRow (K=256 per pass, 3 passes)
                for cn in range(NCH):
                    ps = sg_tile()
                    for hf in range(2):
                        for pi, (xx, ww) in enumerate(
                                ((x8h, wv8h), (x8l, wv8h), (x8h, wv8l))):
                            nc.tensor.matmul(
                                ps[:, hf, :],
                                lhsT=xx[:, :, cn * 128:(cn + 1) * 128],
                                rhs=ww[:, :, hf * 512:(hf + 1) * 512],
                                start=(pi == 0), stop=(pi == 2),
                                perf_mode=PM.DoubleRow,
                            )
                    if cn % 2 == 0:
                        nc.vector.tensor_copy(
                            vt[:, cn, :], ps.rearrange("p g j -> p (g j)"))
                    else:
                        nc.scalar.copy(
                            vt[:, cn, :], ps.rearrange("p g j -> p (g j)"))
                    yield

            return kt, vt, qt, emit()

        def attention(b, h, kt, vt, qt, t2):
            # generator: yields after each score group so the driver can
            # weave projection tiles of the next batch between groups
            yield
            pr, p0 = h // 2, 64 * (h % 2)
            on_pe = BIAS_PE[h]
            bt8 = bt8s[h]
            po = ps_o.tile([128, NQ], f32, tag="po", name=f"po_{b}_{h}")
            e_tiles = []
            for g in range(NCH // GRP):
                sg = sg_tile()
                for j in range(GRP):
                    c = GRP * g + j
                    nc.tensor.matmul(
                        sg[:, j, :NQ],
                        lhsT=kt[p0:p0 + 64, pr, c * 128:(c + 1) * 128],
                        rhs=qt[p0:p0 + 64, pr, :],
                        start=True, stop=(not on_pe),
                    )
                    if on_pe:
                        nc.tensor.matmul(
                            sg[:, j, :NQ],
                            lhsT=identp[:, j, :, :],
                            rhs=bt8[:, GRP * g:GRP * (g + 1), :],
                            start=False, stop=True,
                            perf_mode=PM.DoubleRow,
                        )
                e = e_p.tile([128, GRP, NQ], bf16)
                nc.scalar.activation(e, sg[:, :, :NQ], AF.Exp)
                e_tiles.append(e)
                for j in range(GRP):
                    c = GRP * g + j
                    nc.tensor.matmul(
                        po[:, :NQ],
                        lhsT=vt[:, c, h * 128:(h + 1) * 128],
                        rhs=e[:, j, :],
                        start=(c == 0), stop=(c == NCH - 1),
                    )
                yield
            # softmax denominators
            psm = ps_sum.tile([128, NQ], f32, tag="psm", name=f"psm_{b}_{h}")
            if SUMS_PE[h]:
                for g in range(NCH // GRP):
                    for j in range(GRP):
                        c = GRP * g + j
                        nc.tensor.matmul(
                            psm,
                            lhsT=ones,
                            rhs=e_tiles[g][:, j, :],
                            start=(c == 0), stop=(c == NCH - 1),
                        )
            else:
                t12 = e_tiles[1]
                nc.vector.tensor_tensor(t12, e_tiles[0], e_tiles[1], ALU.add)
                t34 = e_tiles[3]
                nc.vector.tensor_tensor(t34, e_tiles[2], e_tiles[3], ALU.add)
                nc.vector.tensor_tensor(t12, t12, t34, ALU.add)
                nc.vector.tensor_tensor(t12, t12, e_tiles[4], ALU.add)
                esum = esum_p.tile([128, NQ], bf16)
                nc.vector.tensor_tensor(esum, t12[:, 0, :], t12[:, 1, :], ALU.add)
                nc.tensor.matmul(psm, lhsT=ones, rhs=esum, start=True, stop=True)
            rc = rc_p.tile([128, NQ], bf16)
            nc.vector.reciprocal(rc, psm)
            oraw = orw_p.tile([128, NQ], bf16, tag="oraw", name=f"oraw_{b}_{h}")
            nc.vector.tensor_copy(oraw, po)
            oh = oh_p.tile([128, NQ], bf16)
            nc.gpsimd.tensor_tensor(oh, oraw, rc, ALU.mult)
            # hswish: t = min(Relu(o + 3 + bv), 6);  th = (o + bv) * t
            t1 = t1_p.tile([128, NQ], bf16)
            nc.gpsimd.tensor_scalar(t1, oh, bvs3[:, h:h + 1], 0.0, ALU.add, ALU.max)
            nc.gpsimd.tensor_scalar(t1, t1, 6.0, None, ALU.min)
            obv = t1_p.tile([128, NQ], bf16, tag="obv", name=f"obv_{b}_{h}")
            nc.gpsimd.tensor_scalar(obv, oh, bvs[:, h:h + 1], None, ALU.add)
            nc.gpsimd.tensor_tensor(t2[:, h, b % 2, :], obv, t1, ALU.mult)

        def out_proj(b, t2, qcs=range(5)):
            for qc in qcs:
                r0 = (b - 1) * NQ + qc * 128
                ps = sg_tile()
                for dc in range(8):
                    nc.tensor.matmul(
                        ps[:, 0, :OUT],
                        lhsT=t2[:, dc, :, :].rearrange(
                            "p bb q -> p (bb q)")[:, qc * 128:(qc + 1) * 128],
                        rhs=wp[:, dc, :],
                        start=(dc == 0), stop=(dc == 7),
                    )
                ob = ob_p.tile([128, OUT], f32)
                nc.vector.tensor_tensor(ob, ps[:, 0, :OUT], bps, ALU.add)
                nc.sync.dma_start(out_flat[r0:r0 + 128, :], ob)
                yield

        # prologue: batch 0 inputs + weights; proj(0) up to attention-ready
        xt0, xst0, x8h0, x8l0 = dma_x(0, first=True)
        nc.sync.dma_start(identp, a["identp"])
        nc.sync.dma_start(wv8h, a["wv8h"].rearrange("c p j -> p c j"))
        nc.sync.dma_start(bt8s[0], a["bt8"][0])
        nc.sync.dma_start(wv8l, a["wv8l"].rearrange("c p j -> p c j"))
        nc.sync.dma_start(bt8s[1], a["bt8"][1])
        nc.sync.dma_start(ones, a["ones"])
        nc.sync.dma_start(bvs, a["bv"])
        nc.sync.dma_start(bvs3, a["bv3"])

        # warm up the PE p-state during the input-DMA wait: dummy matmuls
        # on a memset scratch tile so the ramp starts at ~0.5us, reaching
        # full clock before the first real projection matmul
        warm = sg_tile()
        for i in range(8):
            nc.tensor.matmul(
                warm[:, 0, :256],
                lhsT=wk[:, 0, 0:128],
                rhs=wk[:, :, 0:128],
                start=True, stop=True,
            )

        kt, vt, qt, gen0 = proj_gen(0, xt0, xst0, x8h0, x8l0)
        for _ in gen0:          # batch 0 proj must fully precede its attention
            pass
        pending = []
        t2 = None
        nxt = None
        for b in range(BPC):
            if b % 2 == 0:
                t2 = hs_p.tile([128, H, 2, NQ], bf16, tag="t2", name=f"t2_{b}")
            # interleave remaining proj tiles (this batch's tail + next batch)
            if b + 1 < BPC:
                xtn, xstn, x8hn, x8ln = dma_x(b + 1, stagger=(b == 0))
                nxt = proj_gen(b + 1, xtn, xstn, x8hn, x8ln)
                pending.append(nxt[3])
            for h in range(H):
                for gi, _ in enumerate(attention(b, h, kt, vt, qt, t2)):
                    for _ in range(2 if gi == 5 else (1 if gi else 0)):
                        while pending:
                            if next(pending[0], "done") == "done":
                                pending.pop(0)
                            else:
                                break
            while pending:
                if next(pending[0], "done") == "done":
                    pending.pop(0)
                else:
                    break
            if pending:
                for _ in pending[0]:
                    pass
                pending.pop(0)
            if b % 2 == 1:
                if b + 1 < BPC:
                    pending.append(out_proj(b, t2))  # interleave with next batch
                else:
                    for _ in out_proj(b, t2):
                        pass
            if nxt is not None:
                kt, vt, qt = nxt[0], nxt[1], nxt[2]
                nxt = None


def build():
    import concourse.mybir as mybir
    import concourse.tile as tile
    from concourse import bacc

    nc = bacc.Bacc("TRN2", target_bir_lowering=False, debug=False)
    f32, bf16 = mybir.dt.float32, mybir.dt.bfloat16
    f8e4 = mybir.dt.float8e4
    a = {}

    def din(name, shape, dt=f32):
        a[name] = nc.dram_tensor(name, shape, dt, kind="ExternalInput").ap()

    f32r = mybir.dt.float32r
    din("xt", [BPC, 2, 128, N], f32r)
    din("xst", [BPC, 2, 128, NQ], f32r)
    din("wkt", [2, 128, 512], f32r)
    din("wvt", [2, 128, 1024], f32r)
    din("x8h", [BPC, 2, 128, N], f8e4)
    din("x8l", [BPC, 2, 128, N], f8e4)
    din("wv8h", [2, 128, 1024], f8e4)
    din("wv8l", [2, 128, 1024], f8e4)
    din("wqt", [2, 128, 512], f32r)
    din("wpt", [8, 128, OUT], bf16)
    din("bq", [128, 4])
    din("bv", [128, H])
    din("bv3", [128, H])
    din("bps", [128, OUT])
    din("bt8", [H, 128, NCH, NQ], f8e4)
    din("ones", [128, 128], bf16)
    din("identp", [128, 2, 2, 128], f8e4)
    out_ap = nc.dram_tensor("out", [BPC, NQ, OUT], f32, kind="ExternalOutput").ap()

    with tile.TileContext(nc) as tc:
        _body(tc, a, out_ap)
    nc.compile()
    return nc


_NC_CACHE = None


def _get_nc():
    global _NC_CACHE
    if _NC_CACHE is None:
        _NC_CACHE = build()
    return _NC_CACHE


def kernel(**inputs):
    from concourse.bass_utils import run_bass_kernel_spmd

    in_maps = _prep(inputs)
    nc = _get_nc()
    res = run_bass_kernel_spmd(nc, in_maps, list(range(NCORES)))
    out = np.concatenate([res.results[i]["out"] for i in range(NCORES)], axis=0)
    return np.ascontiguousarray(out, dtype=np.float32)


if __name__ == "__main__":
    rng = np.random.default_rng(0)
    print("smoke: building bass module...")
    nc = build()
    print("built ok:", sum(len(bb.instructions) for bb in nc.m.functions[0].blocks), "instructions")
